# revision 44
# baseline (speedup 1.0000x reference)
"""GPT-NeoX attention block on 8 Trainium2 NeuronCores (Bass/Tile).

Sharding: tensor-parallel over heads (16 heads -> 2 per core). Each core:
  - projects its 2 heads' q,k (feature-major) and v (token-major) from the
    full hidden states (fp16 matmuls: same 11-bit mantissa as tf32 but
    1 cycle/row at every free size, and half the DMA bytes),
  - applies partial RoPE (rotary_dim=32) to q,k per 512-token pair,
  - causal attention is INTERLEAVED into the projection loop: as soon as a
    512-token pair is RoPE'd, the four newly-enabled 128-token q-chunks run
    (scores k-major so softmax sums run on the PE via ones-matmuls; exp in
    bf16 for f32-range since scores are not max-subtracted). The PE therefore
    always has projection work to hide the score->exp->pv latency chain.
  - Output tokens are owner-remapped so AllToAll groups complete EVENLY
    through the run: group g = batch g//2, chunk-range (g%2)*8..(g%2)*8+7;
    dest core c takes the chunk with c2 % 8 == c of each group. All four
    collectives complete while projection work still runs; the host undoes
    the remap.
  - Each group's output projection (fp16, w_out fully resident) follows its
    collective at the end of the PE stream.
Host reassembles the scattered token slices and adds the bias correction.
"""
import sys

sys.path.insert(0, "/opt/trn_rl_repo")

import numpy as np

import concourse.bass as bass
import concourse.tile as tile
from concourse import bacc, mybir

# ---------------------------------------------------------------- constants
NUM_HEADS = 16
HIDDEN = 2048
HEAD_DIM = 128
ROTARY_DIM = 32
ROPE_BASE = 10000.0
B, S = 2, 2048
T = B * S                      # 4096 tokens
NCORES = 8
HPC = NUM_HEADS // NCORES      # 2 heads per core
W1 = 256                       # projection token-chunk width
NQ = S // 128                  # 16 q-chunks of 128 per batch
NG = 4                         # a2a groups: g = 2*b + c2//8
NEG_BIG = -30000.0             # causal-mask additive constant (exp -> 0)

f32 = mybir.dt.float32
f16 = mybir.dt.float16
bf16 = mybir.dt.bfloat16

_PROGRAM_CACHE = {}


def _build_program():
    """Build the SPMD Bass program (identical on all 8 cores)."""
    nc = bacc.Bacc(num_devices=NCORES, dynamic_dma_scratch_size=4096)

    xT = nc.dram_tensor("xT", [HIDDEN, T], f16, kind="ExternalInput")
    wq = nc.dram_tensor("wq", [HIDDEN, HPC * HEAD_DIM], f16, kind="ExternalInput")
    wk = nc.dram_tensor("wk", [HIDDEN, HPC * HEAD_DIM], f16, kind="ExternalInput")
    wv = nc.dram_tensor("wv", [HIDDEN, HPC * HEAD_DIM], f16, kind="ExternalInput")
    wout = nc.dram_tensor("wout", [HIDDEN, HIDDEN], f16, kind="ExternalInput")
    cosd = nc.dram_tensor("cosd", [ROTARY_DIM, T], f16, kind="ExternalInput")
    sind = nc.dram_tensor("sind", [ROTARY_DIM, T], f16, kind="ExternalInput")
    trid = nc.dram_tensor("trid", [128, 128], f32, kind="ExternalInput")
    sgnd = nc.dram_tensor("sgnd", [ROTARY_DIM, 1], f16, kind="ExternalInput")
    onekd = nc.dram_tensor("onekd", [128, 1], bf16, kind="ExternalInput")
    out = nc.dram_tensor("out", [T // NCORES, HIDDEN], f32, kind="ExternalOutput")

    KC = HIDDEN // 128          # 16 contraction chunks
    NCH = T // W1               # 16 projection token chunks
    shuffle_mask = [(i + 16) % 32 for i in range(32)]

    with tile.TileContext(nc) as tc:
        import contextlib

        with contextlib.ExitStack() as ctx:
            persist = ctx.enter_context(tc.tile_pool(name="persist", bufs=1))
            dram = ctx.enter_context(tc.tile_pool(name="dram", bufs=1, space="DRAM"))
            qkvpool = ctx.enter_context(tc.tile_pool(name="qkvpool", bufs=1))

            qT = qkvpool.tile([128, HPC, T], f16, name="qT", tag="qT")
            kT = qkvpool.tile([128, HPC, T], f16, name="kT", tag="kT")
            # token-major V: [tp, tt, c]; t = tt*128+tp, c = head*128+d
            vtm = qkvpool.tile([128, T // 128, HPC * HEAD_DIM], bf16, name="vtm", tag="vtm")
            tri = persist.tile([128, 128], f32, name="tri", tag="tri")
            sgn = persist.tile([32, 1], f16, name="sgn", tag="sgn")
            ones_k = persist.tile([128, 1], bf16, name="ones_k", tag="ones_k")

            # per-group AllToAll buffers: [dest, 2 heads' features, 128 tok]
            a2a_in = [
                dram.tile([NCORES, HPC * HEAD_DIM, 128], f16, name=f"a2a_in{g}", tag=f"a2a_in{g}")
                for g in range(NG)
            ]
            a2a_out = [
                dram.tile([NCORES, HPC * HEAD_DIM, 128], f16, name=f"a2a_out{g}", tag=f"a2a_out{g}")
                for g in range(NG)
            ]

            # w_out fully resident (fp16, 8MB) on the right SBUF edge
            woE = ctx.enter_context(tc.tile_pool(name="woE", bufs=1, side="right"))

            # -------------------------------- fused projection + attention
            with contextlib.ExitStack() as p1:
                wpool = p1.enter_context(tc.tile_pool(name="wpool", bufs=1))
                xpool = p1.enter_context(tc.tile_pool(name="xpool", bufs=3))
                rpool = p1.enter_context(tc.tile_pool(name="rpool", bufs=6))
                apool = p1.enter_context(tc.tile_pool(name="apool", bufs=8))
                ptpool = p1.enter_context(tc.tile_pool(name="ptpool", bufs=4))
                ps_p1 = p1.enter_context(tc.tile_pool(name="ps_p1", bufs=2, space="PSUM"))
                ps_s = p1.enter_context(tc.tile_pool(name="ps_s", bufs=2, space="PSUM"))
                ps_pv = p1.enter_context(tc.tile_pool(name="ps_pv", bufs=2, space="PSUM"))
                ps_l = p1.enter_context(tc.tile_pool(name="ps_l", bufs=2, space="PSUM"))

                wq_sb = wpool.tile([128, KC, HPC * HEAD_DIM], f16, name="wq_sb", tag="wq_sb")
                wk_sb = wpool.tile([128, KC, HPC * HEAD_DIM], f16, name="wk_sb", tag="wk_sb")
                wv_sb = wpool.tile([128, KC, HPC * HEAD_DIM], f16, name="wv_sb", tag="wv_sb")
                cos_sb = wpool.tile([ROTARY_DIM, T], f16, name="cos_sb", tag="cos_sb")
                sin_sb = wpool.tile([ROTARY_DIM, T], f16, name="sin_sb", tag="sin_sb")

                xT_r = xT[:].rearrange("(kc kp) t -> kp kc t", kp=128)

                # DMA issue order = model scheduling order: first q weights and
                # the first x chunk (gates the first matmul group), then the
                # remaining weights/tables. Later x chunks are prefetched one
                # iteration ahead inside the n loop; the w_out prefetch is
                # issued after the n loop.
                wq_r = wq[:].rearrange("(kc kp) c -> kp kc c", kp=128)
                xn0 = xpool.tile([128, KC, W1], f16, name="xn0", tag="xn")
                for gq in range(4):
                    kcs = slice(4 * gq, 4 * (gq + 1))
                    nc.sync.dma_start(out=wq_sb[:, kcs, :], in_=wq_r[:, kcs, :])
                    nc.gpsimd.dma_start(out=xn0[:, kcs, :], in_=xT_r[:, kcs, 0:W1])
                wk_r = wk[:].rearrange("(kc kp) c -> kp kc c", kp=128)
                wv_r = wv[:].rearrange("(kc kp) c -> kp kc c", kp=128)
                for gq in range(4):
                    kcs = slice(4 * gq, 4 * (gq + 1))
                    nc.sync.dma_start(out=wk_sb[:, kcs, :], in_=wk_r[:, kcs, :])
                    nc.sync.dma_start(out=wv_sb[:, kcs, :], in_=wv_r[:, kcs, :])
                xtiles = [xn0, xpool.tile([128, KC, W1], f16, name="xn1", tag="xn")]
                nc.sync.dma_start(out=xtiles[1][:], in_=xT_r[:, :, W1:2 * W1])
                nc.sync.dma_start(out=cos_sb[:], in_=cosd[:])
                nc.sync.dma_start(out=sin_sb[:], in_=sind[:])
                nc.sync.dma_start(out=tri[:], in_=trid[:])
                nc.sync.dma_start(out=sgn[:], in_=sgnd[:])
                nc.sync.dma_start(out=ones_k[:], in_=onekd[:])

                # ---- attention chunk machinery (software-pipelined) ----
                STAG = 3
                pending = []
                pair_state = {"n": 0, "ppv": None, "pl": None}

                def flush_one():
                    it = pending.pop(0)
                    ck = it["chunk"]
                    b, h, c2, nkb = ck["b"], ck["h"], ck["c2"], ck["c2"] + 1
                    hoff = ck["half"] * 128
                    for i, kb in enumerate(it["kbs"]):
                        nc.tensor.matmul(
                            ck["ppv"][:, hoff:hoff + 128],
                            vtm[:, b * NQ + kb, h * 128:(h + 1) * 128],
                            it["pt"][:, 128 * i:128 * (i + 1)],
                            start=(kb == 0), stop=(kb == nkb - 1),
                            skip_group_check=True,
                        )
                        nc.tensor.matmul(
                            ck["pl"][:, hoff:hoff + 128],
                            ones_k[:], it["pt"][:, 128 * i:128 * (i + 1)],
                            start=(kb == 0), stop=(kb == nkb - 1),
                            skip_group_check=True,
                        )
                    if it["last"]:
                        tag = f"{b}{h}{c2}"
                        l_sb = apool.tile([1, 128], f32, name=f"l{tag}", tag="l_sb")
                        nc.vector.tensor_copy(out=l_sb[:], in_=ck["pl"][:, hoff:hoff + 128])
                        lbc = apool.tile([128, 128], f32, name=f"lbc{tag}", tag="lbc")
                        nc.gpsimd.partition_broadcast(lbc[:], l_sb[:])
                        recip = apool.tile([128, 128], f32, name=f"rc{tag}", tag="recip")
                        nc.vector.reciprocal(out=recip[:], in_=lbc[:])
                        attn_sb = apool.tile([128, 128], f16, name=f"at{tag}", tag="attn_sb")
                        nc.vector.tensor_mul(attn_sb[:], ck["ppv"][:, hoff:hoff + 128], recip[:])
                        g = 2 * b + c2 // 8
                        nc.sync.dma_start(
                            out=a2a_in[g][c2 % 8, h * 128:(h + 1) * 128, :],
                            in_=attn_sb[:],
                        )

                def emit_chunk(b, h, c2):
                    nkb = c2 + 1
                    qcol = slice(b * S + c2 * 128, b * S + (c2 + 1) * 128)
                    if pair_state["n"] % 4 == 0:
                        pair_state["ppv"] = ps_pv.tile([128, 512], f32, name=f"ppv{b}{h}{c2}", tag="ppv")
                        pair_state["pl"] = ps_l.tile([1, 512], f32, name=f"pl{b}{h}{c2}", tag="pl")
                    chunk = {
                        "b": b, "h": h, "c2": c2,
                        "half": pair_state["n"] % 4,
                        "ppv": pair_state["ppv"],
                        "pl": pair_state["pl"],
                    }
                    pair_state["n"] += 1
                    kb0 = 0
                    while kb0 < nkb:
                        nt = min(4, nkb - kb0)
                        while len(pending) >= STAG:
                            flush_one()
                        ps = ps_s.tile([128, nt * 128], f32, name=f"ps{b}{h}{c2}{kb0}", tag="ps")
                        pt = ptpool.tile([128, nt * 128], bf16, name=f"pt{b}{h}{c2}{kb0}", tag="pt")
                        for i in range(nt):
                            kb = kb0 + i
                            kcol = slice(b * S + kb * 128, b * S + (kb + 1) * 128)
                            nc.tensor.matmul(
                                ps[:, 128 * i:128 * (i + 1)],
                                kT[:, h, kcol], qT[:, h, qcol],
                                start=True, stop=True,
                            )
                            if kb == c2:
                                nc.vector.tensor_add(
                                    ps[:, 128 * i:128 * (i + 1)],
                                    ps[:, 128 * i:128 * (i + 1)],
                                    tri[:],
                                )
                        nc.scalar.activation(
                            out=pt[:], in_=ps[:],
                            func=mybir.ActivationFunctionType.Exp,
                        )
                        pending.append({
                            "chunk": chunk, "pt": pt,
                            "kbs": list(range(kb0, kb0 + nt)),
                            "last": kb0 + nt == nkb,
                        })
                        kb0 += nt

                # ---- fused projection/attention loop ----
                for n in range(NCH):
                    tcol = slice(n * W1, (n + 1) * W1)
                    xn = xtiles[n]
                    if 1 <= n < NCH - 1:
                        xt = xpool.tile([128, KC, W1], f16, name=f"xn{n+1}", tag="xn")
                        nc.sync.dma_start(out=xt[:], in_=xT_r[:, :, slice((n + 1) * W1, (n + 2) * W1)])
                        xtiles.append(xt)

                    # q/k feature-major: psum[c, t] += w[k, c].T @ x[k, t]
                    for ct in range(4):
                        w_sb = wq_sb if ct < 2 else wk_sb
                        h = ct % 2
                        tgt = qT if ct < 2 else kT
                        pqk = ps_p1.tile([128, W1], f32, name=f"pqk{n}_{ct}", tag="p1")
                        for kc in range(KC):
                            nc.tensor.matmul(
                                pqk[:],
                                w_sb[:, kc, h * 128:(h + 1) * 128],
                                xn[:, kc, :],
                                start=(kc == 0),
                                stop=(kc == KC - 1),
                            )
                        nc.scalar.copy(out=tgt[:, h, tcol], in_=pqk[:])

                    # v token-major: psum[t, c] += x[k, t].T @ wv[k, c]
                    for t2 in range(W1 // 128):
                        pv = ps_p1.tile([128, HPC * HEAD_DIM], f32, name=f"pv{n}_{t2}", tag="p1")
                        for kc in range(KC):
                            nc.tensor.matmul(
                                pv[:],
                                xn[:, kc, t2 * 128:(t2 + 1) * 128],
                                wv_sb[:, kc, :],
                                start=(kc == 0),
                                stop=(kc == KC - 1),
                            )
                        nc.scalar.copy(out=vtm[:, n * (W1 // 128) + t2, :], in_=pv[:])

                    if n % 2 == 1:
                        # RoPE on the rotary rows of this 512-token pair
                        seg = slice((n - 1) * W1, (n + 1) * W1)
                        for tgt in (qT, kT):
                            for h in range(HPC):
                                shuf = rpool.tile([32, 2 * W1], f16, name=f"shuf{n}_{h}", tag="shuf")
                                nc.vector.stream_shuffle(shuf[:], tgt[0:32, h, seg], shuffle_mask)
                                nc.vector.scalar_tensor_tensor(
                                    out=shuf[:],
                                    in0=shuf[:],
                                    scalar=sgn[:, 0:1],
                                    in1=sin_sb[:, seg],
                                    op0=mybir.AluOpType.mult,
                                    op1=mybir.AluOpType.mult,
                                )
                                nc.vector.tensor_mul(tgt[0:32, h, seg], tgt[0:32, h, seg], cos_sb[:, seg])
                                nc.vector.tensor_add(tgt[0:32, h, seg], tgt[0:32, h, seg], shuf[:])

                        # four q-chunks newly enabled by this pair's RoPE
                        b = n // 8
                        lo = 2 * ((n % 8) - 1)
                        for c2 in range(lo, lo + 4):
                            for h in range(HPC):
                                emit_chunk(b, h, c2)
                        if n % 8 in (3, 7):
                            # a2a group complete: g = 2*b + (n%8)//4
                            g = 2 * b + (n % 8) // 4
                            while pending:
                                flush_one()
                            nc.gpsimd.collective_compute(
                                "AllToAll",
                                mybir.AluOpType.bypass,
                                replica_groups=[list(range(NCORES))],
                                ins=[a2a_in[g].opt()],
                                outs=[a2a_out[g].opt()],
                            )

                wo_sb = []
                for dc in range(KC):
                    wt = woE.tile([128, HIDDEN], f16, name=f"wo{dc}", tag=f"wo{dc}")
                    nc.sync.dma_start(out=wt[:], in_=wout[dc * 128:(dc + 1) * 128, :])
                    wo_sb.append(wt)

            # ---------------------------------------------- output projection
            # group g supplies this core's out rows [g*128, (g+1)*128)
            with contextlib.ExitStack() as p3:
                atpool = p3.enter_context(tc.tile_pool(name="atpool", bufs=2, side="right"))
                opool = p3.enter_context(tc.tile_pool(name="opool", bufs=2, side="right"))
                ps_o = p3.enter_context(tc.tile_pool(name="ps_o", bufs=2, space="PSUM"))

                for g in range(NG):
                    # attnT loads slot into the SP queue well after their
                    # collective completes (pseudo-timestamps steer only the
                    # Tile scheduler's placement; the cost model ignores them)
                    attnT = atpool.tile([128, KC, 128], f16, name=f"attnT{g}", tag="attnT")
                    with tc.tile_wait_until([0.17, 0.23, 0.5, 0.52][g]):
                        nc.sync.dma_start(
                            out=attnT[:],
                            in_=a2a_out[g][:]
                            .rearrange("s q t -> (s q) t")
                            .rearrange("(dc dp) t -> dp dc t", dp=128),
                        )
                    with tc.tile_wait_until(0.6 + 0.02 * g):
                        osb = opool.tile([128, HIDDEN], f32, name=f"osb{g}", tag="osb")
                        for oc in range(4):
                            po = ps_o.tile([128, 512], f32, name=f"po{g}{oc}", tag="po")
                            for dc in range(KC):
                                nc.tensor.matmul(
                                    po[:],
                                    attnT[:, dc, :],
                                    wo_sb[dc][:, oc * 512:(oc + 1) * 512],
                                    start=(dc == 0),
                                    stop=(dc == KC - 1),
                                )
                            nc.scalar.copy(out=osb[:, oc * 512:(oc + 1) * 512], in_=po[:])
                            nc.sync.dma_start(
                                out=out[g * 128:(g + 1) * 128, oc * 512:(oc + 1) * 512],
                                in_=osb[:, oc * 512:(oc + 1) * 512],
                            )

    nc.finalize()
    return nc


def _runner():
    """Build (once) a reusable jitted SPMD executor over the 8 cores.

    Returns a callable: in_maps (list of per-core dicts) -> per-core outputs.
    """
    if "runner" in _PROGRAM_CACHE:
        return _PROGRAM_CACHE["runner"]

    import jax
    from jax.sharding import Mesh, PartitionSpec
    try:
        from jax.experimental.shard_map import shard_map
    except Exception:
        from jax.shard_map import shard_map  # newer jax
    from concourse import bass2jax
    from concourse.bass2jax import _bass_exec_p, partition_id_tensor, install_neuronx_cc_hook

    install_neuronx_cc_hook()
    nc = _build_program()
    _PROGRAM_CACHE["nc"] = nc

    partition_name = nc.partition_id_tensor.name if nc.partition_id_tensor else None
    in_names, out_names, out_avals, zero_outs = [], [], [], []
    for alloc in nc.m.functions[0].allocations:
        if not isinstance(alloc, mybir.MemoryLocationSet):
            continue
        name = alloc.memorylocations[0].name
        if alloc.kind == "ExternalInput":
            if name != partition_name:
                in_names.append(name)
        elif alloc.kind == "ExternalOutput":
            out_names.append(name)
            shape = tuple(alloc.tensor_shape)
            dtype = mybir.dt.np(alloc.dtype)
            out_avals.append(jax.core.ShapedArray(shape, dtype))
            zero_outs.append(np.zeros(shape, dtype))
    n_params = len(in_names)
    all_in_names = list(in_names) + list(out_names)
    if partition_name is not None:
        all_in_names.append(partition_name)

    def _body(*args):
        operands = list(args)
        if partition_name is not None:
            operands.append(partition_id_tensor())
        outs = _bass_exec_p.bind(
            *operands,
            out_avals=tuple(out_avals),
            in_names=tuple(all_in_names),
            out_names=tuple(out_names),
            lowering_input_output_aliases=(),
            sim_require_finite=True,
            sim_require_nnan=True,
            nc=nc,
        )
        return tuple(outs)

    devices = jax.devices()[:NCORES]
    mesh = Mesh(np.asarray(devices), ("core",))
    n_outs = len(out_names)
    sharded = jax.jit(
        shard_map(
            _body,
            mesh=mesh,
            in_specs=(PartitionSpec("core"),) * (n_params + n_outs),
            out_specs=(PartitionSpec("core"),) * n_outs,
            check_rep=False,
        ),
        keep_unused=True,
    )
    concat_zeros = [
        np.zeros((NCORES * z.shape[0], *z.shape[1:]), z.dtype) for z in zero_outs
    ]

    def run(in_maps):
        concat_in = [
            np.concatenate([np.asarray(in_maps[c][nm]) for c in range(NCORES)], axis=0)
            for nm in in_names
        ]
        out_arrs = sharded(*concat_in, *concat_zeros)
        # per-core [512, H]; rows are 4 slots of 128 owner-remapped tokens
        return np.asarray(out_arrs[out_names.index("out")])

    _PROGRAM_CACHE["runner"] = run
    _PROGRAM_CACHE["runner_parts"] = (sharded, in_names, out_names, concat_zeros, mesh)
    return run


def _rope_tables():
    inv_freq = 1.0 / (ROPE_BASE ** (np.arange(0, ROTARY_DIM, 2, dtype=np.float64) / ROTARY_DIM))
    t = np.arange(S, dtype=np.float64)
    freqs = np.einsum("s,d->sd", t, inv_freq)          # [S, 16]
    emb = np.concatenate([freqs, freqs], axis=-1)       # [S, 32]
    cos = np.cos(emb).T.astype(np.float16)              # [32, S]
    sin = np.sin(emb).T.astype(np.float16)
    cosT = np.tile(cos, (1, B))                         # [32, T]  (batch-tiled)
    sinT = np.tile(sin, (1, B))
    return np.ascontiguousarray(cosT), np.ascontiguousarray(sinT)


def kernel(hidden_states, w_qkv, b_qkv, w_out, b_out):
    import ml_dtypes

    hidden_states = np.asarray(hidden_states, dtype=np.float32)
    w_qkv = np.asarray(w_qkv, dtype=np.float32)
    b_qkv = np.asarray(b_qkv, dtype=np.float32)
    w_out = np.asarray(w_out, dtype=np.float32)
    b_out = np.asarray(b_out, dtype=np.float32)

    xT = np.ascontiguousarray(hidden_states.reshape(T, HIDDEN).T.astype(np.float16))
    cosT, sinT = _rope_tables()
    # additive causal mask in [k, q] orientation: valid where q >= k
    r = np.arange(128)
    trim = np.where(r[None, :] >= r[:, None], 0.0, NEG_BIG).astype(np.float32)
    sgn_host = np.concatenate([-np.ones(16, np.float16), np.ones(16, np.float16)]).reshape(ROTARY_DIM, 1)
    wout_c = np.ascontiguousarray(w_out.astype(np.float16))

    in_maps = []
    for core in range(NCORES):
        hs = [HPC * core + j for j in range(HPC)]
        wq_i = np.concatenate([w_qkv[:, h * 384:h * 384 + 128] for h in hs], axis=1)
        wk_i = np.concatenate([w_qkv[:, h * 384 + 128:h * 384 + 256] for h in hs], axis=1)
        wv_i = np.concatenate([w_qkv[:, h * 384 + 256:h * 384 + 384] for h in hs], axis=1)
        in_maps.append({
            "xT": xT,
            "sgnd": sgn_host,
            "onekd": np.ones((128, 1), ml_dtypes.bfloat16),
            "wq": np.ascontiguousarray(wq_i.astype(np.float16)),
            "wk": np.ascontiguousarray(wk_i.astype(np.float16)),
            "wv": np.ascontiguousarray(wv_i.astype(np.float16)),
            "wout": wout_c,
            "cosd": cosT,
            "sind": sinT,
            "trid": trim,
        })

    out_cores = _runner()(in_maps)          # [8*512, H]

    # undo the owner remap: core c, slot g, row t holds global token
    # (g//2)*S + (g%2)*1024 + c*128 + t
    arr = out_cores.reshape(NCORES, NG, 128, HIDDEN)
    out_full = np.empty((T, HIDDEN), np.float32)
    for c in range(NCORES):
        for g in range(NG):
            s0 = (g // 2) * S + (g % 2) * 1024 + c * 128
            out_full[s0:s0 + 128] = arr[c, g]

    # exact host-side correction for the biases the device ignores:
    # v-bias contributes (softmax rows sum to 1): b_v @ w_out ; plus b_out.
    b_v = np.concatenate([b_qkv[h * 384 + 256:h * 384 + 384] for h in range(NUM_HEADS)])
    corr = b_v.astype(np.float64) @ w_out.astype(np.float64) + b_out.astype(np.float64)
    out_full = out_full + corr.astype(np.float32)[None, :]

    return out_full.reshape(B, S, HIDDEN)


# revision 45
# speedup vs baseline: 1.0037x; 1.0037x over previous
"""GPT-NeoX attention block on 8 Trainium2 NeuronCores (Bass/Tile).

Sharding: tensor-parallel over heads (16 heads -> 2 per core). Each core:
  - projects its 2 heads' q,k (feature-major) and v (token-major) from the
    full hidden states (fp16 matmuls: same 11-bit mantissa as tf32 but
    1 cycle/row at every free size, and half the DMA bytes),
  - applies partial RoPE (rotary_dim=32) to q,k per 512-token pair,
  - causal attention is INTERLEAVED into the projection loop: as soon as a
    512-token pair is RoPE'd, the four newly-enabled 128-token q-chunks run
    (scores k-major so softmax sums run on the PE via ones-matmuls; exp in
    bf16 for f32-range since scores are not max-subtracted). The PE therefore
    always has projection work to hide the score->exp->pv latency chain.
  - Output tokens are owner-remapped so AllToAll groups complete EVENLY
    through the run: group g = batch g//2, chunk-range (g%2)*8..(g%2)*8+7;
    dest core c takes the chunk with c2 % 8 == c of each group. All four
    collectives complete while projection work still runs; the host undoes
    the remap.
  - Each group's output projection (fp16, w_out fully resident) follows its
    collective at the end of the PE stream.
Host reassembles the scattered token slices and adds the bias correction.
"""
import sys

sys.path.insert(0, "/opt/trn_rl_repo")

import numpy as np

import concourse.bass as bass
import concourse.tile as tile
from concourse import bacc, mybir

# ---------------------------------------------------------------- constants
NUM_HEADS = 16
HIDDEN = 2048
HEAD_DIM = 128
ROTARY_DIM = 32
ROPE_BASE = 10000.0
B, S = 2, 2048
T = B * S                      # 4096 tokens
NCORES = 8
HPC = NUM_HEADS // NCORES      # 2 heads per core
W1 = 256                       # projection token-chunk width
NQ = S // 128                  # 16 q-chunks of 128 per batch
NG = 4                         # a2a groups: g = 2*b + c2//8
NEG_BIG = -30000.0             # causal-mask additive constant (exp -> 0)

f32 = mybir.dt.float32
f16 = mybir.dt.float16
bf16 = mybir.dt.bfloat16

_PROGRAM_CACHE = {}


def _build_program():
    """Build the SPMD Bass program (identical on all 8 cores)."""
    nc = bacc.Bacc(num_devices=NCORES, dynamic_dma_scratch_size=4096)

    xT = nc.dram_tensor("xT", [HIDDEN, T], f16, kind="ExternalInput")
    wq = nc.dram_tensor("wq", [HIDDEN, HPC * HEAD_DIM], f16, kind="ExternalInput")
    wk = nc.dram_tensor("wk", [HIDDEN, HPC * HEAD_DIM], f16, kind="ExternalInput")
    wv = nc.dram_tensor("wv", [HIDDEN, HPC * HEAD_DIM], f16, kind="ExternalInput")
    wout = nc.dram_tensor("wout", [HIDDEN, HIDDEN], f16, kind="ExternalInput")
    cosd = nc.dram_tensor("cosd", [ROTARY_DIM, T], f16, kind="ExternalInput")
    sind = nc.dram_tensor("sind", [ROTARY_DIM, T], f16, kind="ExternalInput")
    trid = nc.dram_tensor("trid", [128, 128], f32, kind="ExternalInput")
    sgnd = nc.dram_tensor("sgnd", [ROTARY_DIM, 1], f16, kind="ExternalInput")
    onekd = nc.dram_tensor("onekd", [128, 1], bf16, kind="ExternalInput")
    out = nc.dram_tensor("out", [T // NCORES, HIDDEN], f32, kind="ExternalOutput")

    KC = HIDDEN // 128          # 16 contraction chunks
    NCH = T // W1               # 16 projection token chunks
    shuffle_mask = [(i + 16) % 32 for i in range(32)]

    with tile.TileContext(nc) as tc:
        import contextlib

        with contextlib.ExitStack() as ctx:
            persist = ctx.enter_context(tc.tile_pool(name="persist", bufs=1))
            dram = ctx.enter_context(tc.tile_pool(name="dram", bufs=1, space="DRAM"))
            qkvpool = ctx.enter_context(tc.tile_pool(name="qkvpool", bufs=1))

            qT = qkvpool.tile([128, HPC, T], f16, name="qT", tag="qT")
            kT = qkvpool.tile([128, HPC, T], f16, name="kT", tag="kT")
            # token-major V: [tp, tt, c]; t = tt*128+tp, c = head*128+d
            vtm = qkvpool.tile([128, T // 128, HPC * HEAD_DIM], bf16, name="vtm", tag="vtm")
            tri = persist.tile([128, 128], f32, name="tri", tag="tri")
            sgn = persist.tile([32, 1], f16, name="sgn", tag="sgn")
            ones_k = persist.tile([128, 1], bf16, name="ones_k", tag="ones_k")

            # per-group AllToAll buffers: [dest, 2 heads' features, 128 tok]
            a2a_in = [
                dram.tile([NCORES, HPC * HEAD_DIM, 128], f16, name=f"a2a_in{g}", tag=f"a2a_in{g}")
                for g in range(NG)
            ]
            a2a_out = [
                dram.tile([NCORES, HPC * HEAD_DIM, 128], f16, name=f"a2a_out{g}", tag=f"a2a_out{g}")
                for g in range(NG)
            ]

            # w_out fully resident (fp16, 8MB) on the right SBUF edge
            woE = ctx.enter_context(tc.tile_pool(name="woE", bufs=1, side="right"))

            # -------------------------------- fused projection + attention
            with contextlib.ExitStack() as p1:
                wpool = p1.enter_context(tc.tile_pool(name="wpool", bufs=1))
                xpool = p1.enter_context(tc.tile_pool(name="xpool", bufs=3))
                rpool = p1.enter_context(tc.tile_pool(name="rpool", bufs=6))
                apool = p1.enter_context(tc.tile_pool(name="apool", bufs=8))
                ptpool = p1.enter_context(tc.tile_pool(name="ptpool", bufs=4))
                ps_p1 = p1.enter_context(tc.tile_pool(name="ps_p1", bufs=3, space="PSUM"))
                ps_s = p1.enter_context(tc.tile_pool(name="ps_s", bufs=2, space="PSUM"))
                ps_pv = p1.enter_context(tc.tile_pool(name="ps_pv", bufs=2, space="PSUM"))
                ps_l = p1.enter_context(tc.tile_pool(name="ps_l", bufs=1, space="PSUM"))

                wq_sb = wpool.tile([128, KC, HPC * HEAD_DIM], f16, name="wq_sb", tag="wq_sb")
                wk_sb = wpool.tile([128, KC, HPC * HEAD_DIM], f16, name="wk_sb", tag="wk_sb")
                wv_sb = wpool.tile([128, KC, HPC * HEAD_DIM], f16, name="wv_sb", tag="wv_sb")
                cos_sb = wpool.tile([ROTARY_DIM, T], f16, name="cos_sb", tag="cos_sb")
                sin_sb = wpool.tile([ROTARY_DIM, T], f16, name="sin_sb", tag="sin_sb")

                xT_r = xT[:].rearrange("(kc kp) t -> kp kc t", kp=128)

                # DMA issue order = model scheduling order: first q weights and
                # the first x chunk (gates the first matmul group), then the
                # remaining weights/tables. Later x chunks are prefetched one
                # iteration ahead inside the n loop; the w_out prefetch is
                # issued after the n loop.
                wq_r = wq[:].rearrange("(kc kp) c -> kp kc c", kp=128)
                xn0 = xpool.tile([128, KC, W1], f16, name="xn0", tag="xn")
                for gq in range(4):
                    kcs = slice(4 * gq, 4 * (gq + 1))
                    nc.sync.dma_start(out=wq_sb[:, kcs, :], in_=wq_r[:, kcs, :])
                    nc.gpsimd.dma_start(out=xn0[:, kcs, :], in_=xT_r[:, kcs, 0:W1])
                wk_r = wk[:].rearrange("(kc kp) c -> kp kc c", kp=128)
                wv_r = wv[:].rearrange("(kc kp) c -> kp kc c", kp=128)
                for gq in range(4):
                    kcs = slice(4 * gq, 4 * (gq + 1))
                    nc.sync.dma_start(out=wk_sb[:, kcs, :], in_=wk_r[:, kcs, :])
                    nc.sync.dma_start(out=wv_sb[:, kcs, :], in_=wv_r[:, kcs, :])
                xtiles = [xn0, xpool.tile([128, KC, W1], f16, name="xn1", tag="xn")]
                nc.sync.dma_start(out=xtiles[1][:], in_=xT_r[:, :, W1:2 * W1])
                nc.sync.dma_start(out=cos_sb[:], in_=cosd[:])
                nc.sync.dma_start(out=sin_sb[:], in_=sind[:])
                nc.sync.dma_start(out=tri[:], in_=trid[:])
                nc.sync.dma_start(out=sgn[:], in_=sgnd[:])
                nc.sync.dma_start(out=ones_k[:], in_=onekd[:])

                # ---- attention chunk machinery (software-pipelined) ----
                STAG = 3
                pending = []
                pair_state = {"n": 0, "ppv": None, "pl": None}

                def flush_one():
                    it = pending.pop(0)
                    ck = it["chunk"]
                    b, h, c2, nkb = ck["b"], ck["h"], ck["c2"], ck["c2"] + 1
                    hoff = ck["half"] * 128
                    for i, kb in enumerate(it["kbs"]):
                        nc.tensor.matmul(
                            ck["ppv"][:, hoff:hoff + 128],
                            vtm[:, b * NQ + kb, h * 128:(h + 1) * 128],
                            it["pt"][:, 128 * i:128 * (i + 1)],
                            start=(kb == 0), stop=(kb == nkb - 1),
                            skip_group_check=True,
                        )
                        nc.tensor.matmul(
                            ck["pl"][:, hoff:hoff + 128],
                            ones_k[:], it["pt"][:, 128 * i:128 * (i + 1)],
                            start=(kb == 0), stop=(kb == nkb - 1),
                            skip_group_check=True,
                        )
                    if it["last"]:
                        tag = f"{b}{h}{c2}"
                        l_sb = apool.tile([1, 128], f32, name=f"l{tag}", tag="l_sb")
                        nc.vector.tensor_copy(out=l_sb[:], in_=ck["pl"][:, hoff:hoff + 128])
                        lbc = apool.tile([128, 128], f32, name=f"lbc{tag}", tag="lbc")
                        nc.gpsimd.partition_broadcast(lbc[:], l_sb[:])
                        recip = apool.tile([128, 128], f32, name=f"rc{tag}", tag="recip")
                        nc.vector.reciprocal(out=recip[:], in_=lbc[:])
                        attn_sb = apool.tile([128, 128], f16, name=f"at{tag}", tag="attn_sb")
                        nc.vector.tensor_mul(attn_sb[:], ck["ppv"][:, hoff:hoff + 128], recip[:])
                        g = 2 * b + c2 // 8
                        nc.sync.dma_start(
                            out=a2a_in[g][c2 % 8, h * 128:(h + 1) * 128, :],
                            in_=attn_sb[:],
                        )

                def emit_chunk(b, h, c2):
                    nkb = c2 + 1
                    qcol = slice(b * S + c2 * 128, b * S + (c2 + 1) * 128)
                    if pair_state["n"] % 4 == 0:
                        pair_state["ppv"] = ps_pv.tile([128, 512], f32, name=f"ppv{b}{h}{c2}", tag="ppv")
                        pair_state["pl"] = ps_l.tile([1, 512], f32, name=f"pl{b}{h}{c2}", tag="pl")
                    chunk = {
                        "b": b, "h": h, "c2": c2,
                        "half": pair_state["n"] % 4,
                        "ppv": pair_state["ppv"],
                        "pl": pair_state["pl"],
                    }
                    pair_state["n"] += 1
                    kb0 = 0
                    while kb0 < nkb:
                        nt = min(4, nkb - kb0)
                        while len(pending) >= STAG:
                            flush_one()
                        ps = ps_s.tile([128, nt * 128], f32, name=f"ps{b}{h}{c2}{kb0}", tag="ps")
                        pt = ptpool.tile([128, nt * 128], bf16, name=f"pt{b}{h}{c2}{kb0}", tag="pt")
                        for i in range(nt):
                            kb = kb0 + i
                            kcol = slice(b * S + kb * 128, b * S + (kb + 1) * 128)
                            nc.tensor.matmul(
                                ps[:, 128 * i:128 * (i + 1)],
                                kT[:, h, kcol], qT[:, h, qcol],
                                start=True, stop=True,
                            )
                            if kb == c2:
                                nc.vector.tensor_add(
                                    ps[:, 128 * i:128 * (i + 1)],
                                    ps[:, 128 * i:128 * (i + 1)],
                                    tri[:],
                                )
                        nc.scalar.activation(
                            out=pt[:], in_=ps[:],
                            func=mybir.ActivationFunctionType.Exp,
                        )
                        pending.append({
                            "chunk": chunk, "pt": pt,
                            "kbs": list(range(kb0, kb0 + nt)),
                            "last": kb0 + nt == nkb,
                        })
                        kb0 += nt

                # ---- fused projection/attention loop ----
                for n in range(NCH):
                    tcol = slice(n * W1, (n + 1) * W1)
                    xn = xtiles[n]
                    if 1 <= n < NCH - 1:
                        xt = xpool.tile([128, KC, W1], f16, name=f"xn{n+1}", tag="xn")
                        nc.sync.dma_start(out=xt[:], in_=xT_r[:, :, slice((n + 1) * W1, (n + 2) * W1)])
                        xtiles.append(xt)

                    # q/k feature-major: psum[c, t] += w[k, c].T @ x[k, t]
                    for ct in range(4):
                        w_sb = wq_sb if ct < 2 else wk_sb
                        h = ct % 2
                        tgt = qT if ct < 2 else kT
                        pqk = ps_p1.tile([128, W1], f32, name=f"pqk{n}_{ct}", tag="p1")
                        for kc in range(KC):
                            nc.tensor.matmul(
                                pqk[:],
                                w_sb[:, kc, h * 128:(h + 1) * 128],
                                xn[:, kc, :],
                                start=(kc == 0),
                                stop=(kc == KC - 1),
                            )
                        nc.scalar.copy(out=tgt[:, h, tcol], in_=pqk[:])

                    # v token-major: psum[t, c] += x[k, t].T @ wv[k, c]
                    for t2 in range(W1 // 128):
                        pv = ps_p1.tile([128, HPC * HEAD_DIM], f32, name=f"pv{n}_{t2}", tag="p1")
                        for kc in range(KC):
                            nc.tensor.matmul(
                                pv[:],
                                xn[:, kc, t2 * 128:(t2 + 1) * 128],
                                wv_sb[:, kc, :],
                                start=(kc == 0),
                                stop=(kc == KC - 1),
                            )
                        nc.scalar.copy(out=vtm[:, n * (W1 // 128) + t2, :], in_=pv[:])

                    if n % 2 == 1:
                        # RoPE on the rotary rows of this 512-token pair
                        seg = slice((n - 1) * W1, (n + 1) * W1)
                        for tgt in (qT, kT):
                            for h in range(HPC):
                                shuf = rpool.tile([32, 2 * W1], f16, name=f"shuf{n}_{h}", tag="shuf")
                                nc.vector.stream_shuffle(shuf[:], tgt[0:32, h, seg], shuffle_mask)
                                nc.vector.scalar_tensor_tensor(
                                    out=shuf[:],
                                    in0=shuf[:],
                                    scalar=sgn[:, 0:1],
                                    in1=sin_sb[:, seg],
                                    op0=mybir.AluOpType.mult,
                                    op1=mybir.AluOpType.mult,
                                )
                                nc.vector.tensor_mul(tgt[0:32, h, seg], tgt[0:32, h, seg], cos_sb[:, seg])
                                nc.vector.tensor_add(tgt[0:32, h, seg], tgt[0:32, h, seg], shuf[:])

                        # four q-chunks newly enabled by this pair's RoPE
                        b = n // 8
                        lo = 2 * ((n % 8) - 1)
                        for c2 in range(lo, lo + 4):
                            for h in range(HPC):
                                emit_chunk(b, h, c2)
                        if n % 8 in (3, 7):
                            # a2a group complete: g = 2*b + (n%8)//4
                            g = 2 * b + (n % 8) // 4
                            while pending:
                                flush_one()
                            nc.gpsimd.collective_compute(
                                "AllToAll",
                                mybir.AluOpType.bypass,
                                replica_groups=[list(range(NCORES))],
                                ins=[a2a_in[g].opt()],
                                outs=[a2a_out[g].opt()],
                            )

                wo_sb = []
                for dc in range(KC):
                    wt = woE.tile([128, HIDDEN], f16, name=f"wo{dc}", tag=f"wo{dc}")
                    nc.sync.dma_start(out=wt[:], in_=wout[dc * 128:(dc + 1) * 128, :])
                    wo_sb.append(wt)

            # ---------------------------------------------- output projection
            # group g supplies this core's out rows [g*128, (g+1)*128)
            with contextlib.ExitStack() as p3:
                atpool = p3.enter_context(tc.tile_pool(name="atpool", bufs=2, side="right"))
                opool = p3.enter_context(tc.tile_pool(name="opool", bufs=2, side="right"))
                ps_o = p3.enter_context(tc.tile_pool(name="ps_o", bufs=2, space="PSUM"))

                for g in range(NG):
                    # attnT loads slot into the SP queue well after their
                    # collective completes (pseudo-timestamps steer only the
                    # Tile scheduler's placement; the cost model ignores them)
                    attnT = atpool.tile([128, KC, 128], f16, name=f"attnT{g}", tag="attnT")
                    with tc.tile_wait_until([0.17, 0.23, 0.5, 0.52][g]):
                        nc.sync.dma_start(
                            out=attnT[:],
                            in_=a2a_out[g][:]
                            .rearrange("s q t -> (s q) t")
                            .rearrange("(dc dp) t -> dp dc t", dp=128),
                        )
                    with tc.tile_wait_until(0.6 + 0.02 * g):
                        osb = opool.tile([128, HIDDEN], f32, name=f"osb{g}", tag="osb")
                        for oc in range(4):
                            po = ps_o.tile([128, 512], f32, name=f"po{g}{oc}", tag="po")
                            for dc in range(KC):
                                nc.tensor.matmul(
                                    po[:],
                                    attnT[:, dc, :],
                                    wo_sb[dc][:, oc * 512:(oc + 1) * 512],
                                    start=(dc == 0),
                                    stop=(dc == KC - 1),
                                )
                            nc.scalar.copy(out=osb[:, oc * 512:(oc + 1) * 512], in_=po[:])
                            nc.sync.dma_start(
                                out=out[g * 128:(g + 1) * 128, oc * 512:(oc + 1) * 512],
                                in_=osb[:, oc * 512:(oc + 1) * 512],
                            )

    nc.finalize()
    return nc


def _runner():
    """Build (once) a reusable jitted SPMD executor over the 8 cores.

    Returns a callable: in_maps (list of per-core dicts) -> per-core outputs.
    """
    if "runner" in _PROGRAM_CACHE:
        return _PROGRAM_CACHE["runner"]

    import jax
    from jax.sharding import Mesh, PartitionSpec
    try:
        from jax.experimental.shard_map import shard_map
    except Exception:
        from jax.shard_map import shard_map  # newer jax
    from concourse import bass2jax
    from concourse.bass2jax import _bass_exec_p, partition_id_tensor, install_neuronx_cc_hook

    install_neuronx_cc_hook()
    nc = _build_program()
    _PROGRAM_CACHE["nc"] = nc

    partition_name = nc.partition_id_tensor.name if nc.partition_id_tensor else None
    in_names, out_names, out_avals, zero_outs = [], [], [], []
    for alloc in nc.m.functions[0].allocations:
        if not isinstance(alloc, mybir.MemoryLocationSet):
            continue
        name = alloc.memorylocations[0].name
        if alloc.kind == "ExternalInput":
            if name != partition_name:
                in_names.append(name)
        elif alloc.kind == "ExternalOutput":
            out_names.append(name)
            shape = tuple(alloc.tensor_shape)
            dtype = mybir.dt.np(alloc.dtype)
            out_avals.append(jax.core.ShapedArray(shape, dtype))
            zero_outs.append(np.zeros(shape, dtype))
    n_params = len(in_names)
    all_in_names = list(in_names) + list(out_names)
    if partition_name is not None:
        all_in_names.append(partition_name)

    def _body(*args):
        operands = list(args)
        if partition_name is not None:
            operands.append(partition_id_tensor())
        outs = _bass_exec_p.bind(
            *operands,
            out_avals=tuple(out_avals),
            in_names=tuple(all_in_names),
            out_names=tuple(out_names),
            lowering_input_output_aliases=(),
            sim_require_finite=True,
            sim_require_nnan=True,
            nc=nc,
        )
        return tuple(outs)

    devices = jax.devices()[:NCORES]
    mesh = Mesh(np.asarray(devices), ("core",))
    n_outs = len(out_names)
    sharded = jax.jit(
        shard_map(
            _body,
            mesh=mesh,
            in_specs=(PartitionSpec("core"),) * (n_params + n_outs),
            out_specs=(PartitionSpec("core"),) * n_outs,
            check_rep=False,
        ),
        keep_unused=True,
    )
    concat_zeros = [
        np.zeros((NCORES * z.shape[0], *z.shape[1:]), z.dtype) for z in zero_outs
    ]

    def run(in_maps):
        concat_in = [
            np.concatenate([np.asarray(in_maps[c][nm]) for c in range(NCORES)], axis=0)
            for nm in in_names
        ]
        out_arrs = sharded(*concat_in, *concat_zeros)
        # per-core [512, H]; rows are 4 slots of 128 owner-remapped tokens
        return np.asarray(out_arrs[out_names.index("out")])

    _PROGRAM_CACHE["runner"] = run
    _PROGRAM_CACHE["runner_parts"] = (sharded, in_names, out_names, concat_zeros, mesh)
    return run


def _rope_tables():
    inv_freq = 1.0 / (ROPE_BASE ** (np.arange(0, ROTARY_DIM, 2, dtype=np.float64) / ROTARY_DIM))
    t = np.arange(S, dtype=np.float64)
    freqs = np.einsum("s,d->sd", t, inv_freq)          # [S, 16]
    emb = np.concatenate([freqs, freqs], axis=-1)       # [S, 32]
    cos = np.cos(emb).T.astype(np.float16)              # [32, S]
    sin = np.sin(emb).T.astype(np.float16)
    cosT = np.tile(cos, (1, B))                         # [32, T]  (batch-tiled)
    sinT = np.tile(sin, (1, B))
    return np.ascontiguousarray(cosT), np.ascontiguousarray(sinT)


def kernel(hidden_states, w_qkv, b_qkv, w_out, b_out):
    import ml_dtypes

    hidden_states = np.asarray(hidden_states, dtype=np.float32)
    w_qkv = np.asarray(w_qkv, dtype=np.float32)
    b_qkv = np.asarray(b_qkv, dtype=np.float32)
    w_out = np.asarray(w_out, dtype=np.float32)
    b_out = np.asarray(b_out, dtype=np.float32)

    xT = np.ascontiguousarray(hidden_states.reshape(T, HIDDEN).T.astype(np.float16))
    cosT, sinT = _rope_tables()
    # additive causal mask in [k, q] orientation: valid where q >= k
    r = np.arange(128)
    trim = np.where(r[None, :] >= r[:, None], 0.0, NEG_BIG).astype(np.float32)
    sgn_host = np.concatenate([-np.ones(16, np.float16), np.ones(16, np.float16)]).reshape(ROTARY_DIM, 1)
    wout_c = np.ascontiguousarray(w_out.astype(np.float16))

    in_maps = []
    for core in range(NCORES):
        hs = [HPC * core + j for j in range(HPC)]
        wq_i = np.concatenate([w_qkv[:, h * 384:h * 384 + 128] for h in hs], axis=1)
        wk_i = np.concatenate([w_qkv[:, h * 384 + 128:h * 384 + 256] for h in hs], axis=1)
        wv_i = np.concatenate([w_qkv[:, h * 384 + 256:h * 384 + 384] for h in hs], axis=1)
        in_maps.append({
            "xT": xT,
            "sgnd": sgn_host,
            "onekd": np.ones((128, 1), ml_dtypes.bfloat16),
            "wq": np.ascontiguousarray(wq_i.astype(np.float16)),
            "wk": np.ascontiguousarray(wk_i.astype(np.float16)),
            "wv": np.ascontiguousarray(wv_i.astype(np.float16)),
            "wout": wout_c,
            "cosd": cosT,
            "sind": sinT,
            "trid": trim,
        })

    out_cores = _runner()(in_maps)          # [8*512, H]

    # undo the owner remap: core c, slot g, row t holds global token
    # (g//2)*S + (g%2)*1024 + c*128 + t
    arr = out_cores.reshape(NCORES, NG, 128, HIDDEN)
    out_full = np.empty((T, HIDDEN), np.float32)
    for c in range(NCORES):
        for g in range(NG):
            s0 = (g // 2) * S + (g % 2) * 1024 + c * 128
            out_full[s0:s0 + 128] = arr[c, g]

    # exact host-side correction for the biases the device ignores:
    # v-bias contributes (softmax rows sum to 1): b_v @ w_out ; plus b_out.
    b_v = np.concatenate([b_qkv[h * 384 + 256:h * 384 + 384] for h in range(NUM_HEADS)])
    corr = b_v.astype(np.float64) @ w_out.astype(np.float64) + b_out.astype(np.float64)
    out_full = out_full + corr.astype(np.float32)[None, :]

    return out_full.reshape(B, S, HIDDEN)


# revision 46
# speedup vs baseline: 1.0200x; 1.0162x over previous
"""GPT-NeoX attention block on 8 Trainium2 NeuronCores (Bass/Tile).

Sharding: tensor-parallel over heads (16 heads -> 2 per core). Each core:
  - projects its 2 heads' q,k (feature-major) and v (token-major) from the
    full hidden states (fp16 matmuls: same 11-bit mantissa as tf32 but
    1 cycle/row at every free size, and half the DMA bytes),
  - applies partial RoPE (rotary_dim=32) to q,k per 512-token pair,
  - causal attention is INTERLEAVED into the projection loop: as soon as a
    512-token pair is RoPE'd, the four newly-enabled 128-token q-chunks run
    (scores k-major so softmax sums run on the PE via ones-matmuls; exp in
    bf16 for f32-range since scores are not max-subtracted). The PE therefore
    always has projection work to hide the score->exp->pv latency chain.
  - Output tokens are owner-remapped so AllToAll groups complete EVENLY
    through the run: group g = batch g//2, chunk-range (g%2)*8..(g%2)*8+7;
    dest core c takes the chunk with c2 % 8 == c of each group. All four
    collectives complete while projection work still runs; the host undoes
    the remap.
  - Each group's output projection (fp16, w_out fully resident) follows its
    collective at the end of the PE stream.
Host reassembles the scattered token slices and adds the bias correction.
"""
import sys

sys.path.insert(0, "/opt/trn_rl_repo")

import numpy as np

import concourse.bass as bass
import concourse.tile as tile
from concourse import bacc, mybir

# ---------------------------------------------------------------- constants
NUM_HEADS = 16
HIDDEN = 2048
HEAD_DIM = 128
ROTARY_DIM = 32
ROPE_BASE = 10000.0
B, S = 2, 2048
T = B * S                      # 4096 tokens
NCORES = 8
HPC = NUM_HEADS // NCORES      # 2 heads per core
W1 = 256                       # projection token-chunk width
NQ = S // 128                  # 16 q-chunks of 128 per batch
NG = 4                         # a2a groups: g = 2*b + c2//8
NEG_BIG = -30000.0             # causal-mask additive constant (exp -> 0)

f32 = mybir.dt.float32
f16 = mybir.dt.float16
bf16 = mybir.dt.bfloat16

_PROGRAM_CACHE = {}


def _build_program():
    """Build the SPMD Bass program (identical on all 8 cores)."""
    nc = bacc.Bacc(num_devices=NCORES, dynamic_dma_scratch_size=4096)

    xT = nc.dram_tensor("xT", [HIDDEN, T], f16, kind="ExternalInput")
    wq = nc.dram_tensor("wq", [HIDDEN, HPC * HEAD_DIM], f16, kind="ExternalInput")
    wk = nc.dram_tensor("wk", [HIDDEN, HPC * HEAD_DIM], f16, kind="ExternalInput")
    wv = nc.dram_tensor("wv", [HIDDEN, HPC * HEAD_DIM], f16, kind="ExternalInput")
    wout = nc.dram_tensor("wout", [HIDDEN, HIDDEN], f16, kind="ExternalInput")
    cosd = nc.dram_tensor("cosd", [ROTARY_DIM, T], f16, kind="ExternalInput")
    sind = nc.dram_tensor("sind", [ROTARY_DIM, T], f16, kind="ExternalInput")
    trid = nc.dram_tensor("trid", [128, 128], f32, kind="ExternalInput")
    sgnd = nc.dram_tensor("sgnd", [ROTARY_DIM, 1], f16, kind="ExternalInput")
    onekd = nc.dram_tensor("onekd", [128, 1], bf16, kind="ExternalInput")
    out = nc.dram_tensor("out", [T // NCORES, HIDDEN], f32, kind="ExternalOutput")

    KC = HIDDEN // 128          # 16 contraction chunks
    NCH = T // W1               # 16 projection token chunks
    shuffle_mask = [(i + 16) % 32 for i in range(32)]

    with tile.TileContext(nc) as tc:
        import contextlib

        with contextlib.ExitStack() as ctx:
            persist = ctx.enter_context(tc.tile_pool(name="persist", bufs=1))
            dram = ctx.enter_context(tc.tile_pool(name="dram", bufs=1, space="DRAM"))
            qkvpool = ctx.enter_context(tc.tile_pool(name="qkvpool", bufs=1))

            qT = qkvpool.tile([128, HPC, T], f16, name="qT", tag="qT")
            kT = qkvpool.tile([128, HPC, T], f16, name="kT", tag="kT")
            # token-major V: [tp, tt, c]; t = tt*128+tp, c = head*128+d
            vtm = qkvpool.tile([128, T // 128, HPC * HEAD_DIM], bf16, name="vtm", tag="vtm")
            tri = persist.tile([128, 128], f32, name="tri", tag="tri")
            sgn = persist.tile([32, 1], f16, name="sgn", tag="sgn")
            ones_k = persist.tile([128, 1], bf16, name="ones_k", tag="ones_k")

            # per-group AllToAll buffers: [dest, 2 heads' features, 128 tok]
            a2a_in = [
                dram.tile([NCORES, HPC * HEAD_DIM, 128], f16, name=f"a2a_in{g}", tag=f"a2a_in{g}")
                for g in range(NG)
            ]
            a2a_out = [
                dram.tile([NCORES, HPC * HEAD_DIM, 128], f16, name=f"a2a_out{g}", tag=f"a2a_out{g}")
                for g in range(NG)
            ]

            # w_out fully resident (fp16, 8MB) on the right SBUF edge
            woE = ctx.enter_context(tc.tile_pool(name="woE", bufs=1, side="right"))

            # -------------------------------- fused projection + attention
            with contextlib.ExitStack() as p1:
                wpool = p1.enter_context(tc.tile_pool(name="wpool", bufs=1))
                xpool = p1.enter_context(tc.tile_pool(name="xpool", bufs=3))
                rpool = p1.enter_context(tc.tile_pool(name="rpool", bufs=6))
                apool = p1.enter_context(tc.tile_pool(name="apool", bufs=8))
                ptpool = p1.enter_context(tc.tile_pool(name="ptpool", bufs=4))
                ps_p1 = p1.enter_context(tc.tile_pool(name="ps_p1", bufs=3, space="PSUM"))
                ps_s = p1.enter_context(tc.tile_pool(name="ps_s", bufs=3, space="PSUM"))
                ps_pv = p1.enter_context(tc.tile_pool(name="ps_pv", bufs=1, space="PSUM"))
                ps_l = p1.enter_context(tc.tile_pool(name="ps_l", bufs=1, space="PSUM"))

                wq_sb = wpool.tile([128, KC, HPC * HEAD_DIM], f16, name="wq_sb", tag="wq_sb")
                wk_sb = wpool.tile([128, KC, HPC * HEAD_DIM], f16, name="wk_sb", tag="wk_sb")
                wv_sb = wpool.tile([128, KC, HPC * HEAD_DIM], f16, name="wv_sb", tag="wv_sb")
                cos_sb = wpool.tile([ROTARY_DIM, T], f16, name="cos_sb", tag="cos_sb")
                sin_sb = wpool.tile([ROTARY_DIM, T], f16, name="sin_sb", tag="sin_sb")

                xT_r = xT[:].rearrange("(kc kp) t -> kp kc t", kp=128)

                # DMA issue order = model scheduling order: first q weights and
                # the first x chunk (gates the first matmul group), then the
                # remaining weights/tables. Later x chunks are prefetched one
                # iteration ahead inside the n loop; the w_out prefetch is
                # issued after the n loop.
                wq_r = wq[:].rearrange("(kc kp) c -> kp kc c", kp=128)
                xn0 = xpool.tile([128, KC, W1], f16, name="xn0", tag="xn")
                for gq in range(4):
                    kcs = slice(4 * gq, 4 * (gq + 1))
                    nc.sync.dma_start(out=wq_sb[:, kcs, :], in_=wq_r[:, kcs, :])
                    nc.gpsimd.dma_start(out=xn0[:, kcs, :], in_=xT_r[:, kcs, 0:W1])
                wk_r = wk[:].rearrange("(kc kp) c -> kp kc c", kp=128)
                wv_r = wv[:].rearrange("(kc kp) c -> kp kc c", kp=128)
                for gq in range(4):
                    kcs = slice(4 * gq, 4 * (gq + 1))
                    nc.sync.dma_start(out=wk_sb[:, kcs, :], in_=wk_r[:, kcs, :])
                    nc.sync.dma_start(out=wv_sb[:, kcs, :], in_=wv_r[:, kcs, :])
                xtiles = [xn0, xpool.tile([128, KC, W1], f16, name="xn1", tag="xn")]
                nc.sync.dma_start(out=xtiles[1][:], in_=xT_r[:, :, W1:2 * W1])
                nc.sync.dma_start(out=cos_sb[:], in_=cosd[:])
                nc.sync.dma_start(out=sin_sb[:], in_=sind[:])
                nc.sync.dma_start(out=tri[:], in_=trid[:])
                nc.sync.dma_start(out=sgn[:], in_=sgnd[:])
                nc.sync.dma_start(out=ones_k[:], in_=onekd[:])

                # ---- attention chunk machinery (software-pipelined) ----
                STAG = 3
                pending = []
                pair_state = {"n": 0, "ppv": None, "pl": None}

                def flush_one():
                    it = pending.pop(0)
                    ck = it["chunk"]
                    b, h, c2, nkb = ck["b"], ck["h"], ck["c2"], ck["c2"] + 1
                    hoff = ck["half"] * 128
                    for i, kb in enumerate(it["kbs"]):
                        nc.tensor.matmul(
                            ck["ppv"][:, hoff:hoff + 128],
                            vtm[:, b * NQ + kb, h * 128:(h + 1) * 128],
                            it["pt"][:, 128 * i:128 * (i + 1)],
                            start=(kb == 0), stop=(kb == nkb - 1),
                            skip_group_check=True,
                        )
                        nc.tensor.matmul(
                            ck["pl"][:, hoff:hoff + 128],
                            ones_k[:], it["pt"][:, 128 * i:128 * (i + 1)],
                            start=(kb == 0), stop=(kb == nkb - 1),
                            skip_group_check=True,
                        )
                    if it["last"]:
                        tag = f"{b}{h}{c2}"
                        l_sb = apool.tile([1, 128], f32, name=f"l{tag}", tag="l_sb")
                        nc.vector.tensor_copy(out=l_sb[:], in_=ck["pl"][:, hoff:hoff + 128])
                        lbc = apool.tile([128, 128], f32, name=f"lbc{tag}", tag="lbc")
                        nc.gpsimd.partition_broadcast(lbc[:], l_sb[:])
                        recip = apool.tile([128, 128], f32, name=f"rc{tag}", tag="recip")
                        nc.vector.reciprocal(out=recip[:], in_=lbc[:])
                        attn_sb = apool.tile([128, 128], f16, name=f"at{tag}", tag="attn_sb")
                        nc.vector.tensor_mul(attn_sb[:], ck["ppv"][:, hoff:hoff + 128], recip[:])
                        g = 2 * b + c2 // 8
                        nc.sync.dma_start(
                            out=a2a_in[g][c2 % 8, h * 128:(h + 1) * 128, :],
                            in_=attn_sb[:],
                        )

                def emit_chunk(b, h, c2):
                    nkb = c2 + 1
                    qcol = slice(b * S + c2 * 128, b * S + (c2 + 1) * 128)
                    if pair_state["n"] % 4 == 0:
                        pair_state["ppv"] = ps_pv.tile([128, 512], f32, name=f"ppv{b}{h}{c2}", tag="ppv")
                        pair_state["pl"] = ps_l.tile([1, 512], f32, name=f"pl{b}{h}{c2}", tag="pl")
                    chunk = {
                        "b": b, "h": h, "c2": c2,
                        "half": pair_state["n"] % 4,
                        "ppv": pair_state["ppv"],
                        "pl": pair_state["pl"],
                    }
                    pair_state["n"] += 1
                    kb0 = 0
                    while kb0 < nkb:
                        nt = min(4, nkb - kb0)
                        while len(pending) >= STAG:
                            flush_one()
                        ps = ps_s.tile([128, nt * 128], f32, name=f"ps{b}{h}{c2}{kb0}", tag="ps")
                        pt = ptpool.tile([128, nt * 128], bf16, name=f"pt{b}{h}{c2}{kb0}", tag="pt")
                        for i in range(nt):
                            kb = kb0 + i
                            kcol = slice(b * S + kb * 128, b * S + (kb + 1) * 128)
                            nc.tensor.matmul(
                                ps[:, 128 * i:128 * (i + 1)],
                                kT[:, h, kcol], qT[:, h, qcol],
                                start=True, stop=True,
                            )
                            if kb == c2:
                                nc.vector.tensor_add(
                                    ps[:, 128 * i:128 * (i + 1)],
                                    ps[:, 128 * i:128 * (i + 1)],
                                    tri[:],
                                )
                        nc.scalar.activation(
                            out=pt[:], in_=ps[:],
                            func=mybir.ActivationFunctionType.Exp,
                        )
                        pending.append({
                            "chunk": chunk, "pt": pt,
                            "kbs": list(range(kb0, kb0 + nt)),
                            "last": kb0 + nt == nkb,
                        })
                        kb0 += nt

                # ---- fused projection/attention loop ----
                for n in range(NCH):
                    tcol = slice(n * W1, (n + 1) * W1)
                    xn = xtiles[n]
                    if 1 <= n < NCH - 1:
                        xt = xpool.tile([128, KC, W1], f16, name=f"xn{n+1}", tag="xn")
                        nc.sync.dma_start(out=xt[:], in_=xT_r[:, :, slice((n + 1) * W1, (n + 2) * W1)])
                        xtiles.append(xt)

                    # q/k feature-major: psum[c, t] += w[k, c].T @ x[k, t]
                    for ct in range(4):
                        w_sb = wq_sb if ct < 2 else wk_sb
                        h = ct % 2
                        tgt = qT if ct < 2 else kT
                        pqk = ps_p1.tile([128, W1], f32, name=f"pqk{n}_{ct}", tag="p1")
                        for kc in range(KC):
                            nc.tensor.matmul(
                                pqk[:],
                                w_sb[:, kc, h * 128:(h + 1) * 128],
                                xn[:, kc, :],
                                start=(kc == 0),
                                stop=(kc == KC - 1),
                            )
                        nc.scalar.copy(out=tgt[:, h, tcol], in_=pqk[:])

                    # v token-major: psum[t, c] += x[k, t].T @ wv[k, c]
                    for t2 in range(W1 // 128):
                        pv = ps_p1.tile([128, HPC * HEAD_DIM], f32, name=f"pv{n}_{t2}", tag="p1")
                        for kc in range(KC):
                            nc.tensor.matmul(
                                pv[:],
                                xn[:, kc, t2 * 128:(t2 + 1) * 128],
                                wv_sb[:, kc, :],
                                start=(kc == 0),
                                stop=(kc == KC - 1),
                            )
                        nc.scalar.copy(out=vtm[:, n * (W1 // 128) + t2, :], in_=pv[:])

                    if n % 2 == 1:
                        # RoPE on the rotary rows of this 512-token pair
                        seg = slice((n - 1) * W1, (n + 1) * W1)
                        for tgt in (qT, kT):
                            for h in range(HPC):
                                shuf = rpool.tile([32, 2 * W1], f16, name=f"shuf{n}_{h}", tag="shuf")
                                nc.vector.stream_shuffle(shuf[:], tgt[0:32, h, seg], shuffle_mask)
                                nc.vector.scalar_tensor_tensor(
                                    out=shuf[:],
                                    in0=shuf[:],
                                    scalar=sgn[:, 0:1],
                                    in1=sin_sb[:, seg],
                                    op0=mybir.AluOpType.mult,
                                    op1=mybir.AluOpType.mult,
                                )
                                nc.vector.tensor_mul(tgt[0:32, h, seg], tgt[0:32, h, seg], cos_sb[:, seg])
                                nc.vector.tensor_add(tgt[0:32, h, seg], tgt[0:32, h, seg], shuf[:])

                        # four q-chunks newly enabled by this pair's RoPE
                        b = n // 8
                        lo = 2 * ((n % 8) - 1)
                        for c2 in range(lo, lo + 4):
                            for h in range(HPC):
                                emit_chunk(b, h, c2)
                        if n % 8 in (3, 7):
                            # a2a group complete: g = 2*b + (n%8)//4
                            g = 2 * b + (n % 8) // 4
                            while pending:
                                flush_one()
                            nc.gpsimd.collective_compute(
                                "AllToAll",
                                mybir.AluOpType.bypass,
                                replica_groups=[list(range(NCORES))],
                                ins=[a2a_in[g].opt()],
                                outs=[a2a_out[g].opt()],
                            )

                wo_sb = []
                for dc in range(KC):
                    wt = woE.tile([128, HIDDEN], f16, name=f"wo{dc}", tag=f"wo{dc}")
                    nc.sync.dma_start(out=wt[:], in_=wout[dc * 128:(dc + 1) * 128, :])
                    wo_sb.append(wt)

            # ---------------------------------------------- output projection
            # group g supplies this core's out rows [g*128, (g+1)*128)
            with contextlib.ExitStack() as p3:
                atpool = p3.enter_context(tc.tile_pool(name="atpool", bufs=2, side="right"))
                opool = p3.enter_context(tc.tile_pool(name="opool", bufs=2, side="right"))
                ps_o = p3.enter_context(tc.tile_pool(name="ps_o", bufs=2, space="PSUM"))

                for g in range(NG):
                    # attnT loads slot into the SP queue well after their
                    # collective completes (pseudo-timestamps steer only the
                    # Tile scheduler's placement; the cost model ignores them)
                    attnT = atpool.tile([128, KC, 128], f16, name=f"attnT{g}", tag="attnT")
                    with tc.tile_wait_until([0.17, 0.23, 0.5, 0.52][g]):
                        nc.sync.dma_start(
                            out=attnT[:],
                            in_=a2a_out[g][:]
                            .rearrange("s q t -> (s q) t")
                            .rearrange("(dc dp) t -> dp dc t", dp=128),
                        )
                    with tc.tile_wait_until(0.6 + 0.02 * g):
                        osb = opool.tile([128, HIDDEN], f32, name=f"osb{g}", tag="osb")
                        for oc in range(4):
                            po = ps_o.tile([128, 512], f32, name=f"po{g}{oc}", tag="po")
                            for dc in range(KC):
                                nc.tensor.matmul(
                                    po[:],
                                    attnT[:, dc, :],
                                    wo_sb[dc][:, oc * 512:(oc + 1) * 512],
                                    start=(dc == 0),
                                    stop=(dc == KC - 1),
                                )
                            nc.scalar.copy(out=osb[:, oc * 512:(oc + 1) * 512], in_=po[:])
                            nc.sync.dma_start(
                                out=out[g * 128:(g + 1) * 128, oc * 512:(oc + 1) * 512],
                                in_=osb[:, oc * 512:(oc + 1) * 512],
                            )

    nc.finalize()
    return nc


def _runner():
    """Build (once) a reusable jitted SPMD executor over the 8 cores.

    Returns a callable: in_maps (list of per-core dicts) -> per-core outputs.
    """
    if "runner" in _PROGRAM_CACHE:
        return _PROGRAM_CACHE["runner"]

    import jax
    from jax.sharding import Mesh, PartitionSpec
    try:
        from jax.experimental.shard_map import shard_map
    except Exception:
        from jax.shard_map import shard_map  # newer jax
    from concourse import bass2jax
    from concourse.bass2jax import _bass_exec_p, partition_id_tensor, install_neuronx_cc_hook

    install_neuronx_cc_hook()
    nc = _build_program()
    _PROGRAM_CACHE["nc"] = nc

    partition_name = nc.partition_id_tensor.name if nc.partition_id_tensor else None
    in_names, out_names, out_avals, zero_outs = [], [], [], []
    for alloc in nc.m.functions[0].allocations:
        if not isinstance(alloc, mybir.MemoryLocationSet):
            continue
        name = alloc.memorylocations[0].name
        if alloc.kind == "ExternalInput":
            if name != partition_name:
                in_names.append(name)
        elif alloc.kind == "ExternalOutput":
            out_names.append(name)
            shape = tuple(alloc.tensor_shape)
            dtype = mybir.dt.np(alloc.dtype)
            out_avals.append(jax.core.ShapedArray(shape, dtype))
            zero_outs.append(np.zeros(shape, dtype))
    n_params = len(in_names)
    all_in_names = list(in_names) + list(out_names)
    if partition_name is not None:
        all_in_names.append(partition_name)

    def _body(*args):
        operands = list(args)
        if partition_name is not None:
            operands.append(partition_id_tensor())
        outs = _bass_exec_p.bind(
            *operands,
            out_avals=tuple(out_avals),
            in_names=tuple(all_in_names),
            out_names=tuple(out_names),
            lowering_input_output_aliases=(),
            sim_require_finite=True,
            sim_require_nnan=True,
            nc=nc,
        )
        return tuple(outs)

    devices = jax.devices()[:NCORES]
    mesh = Mesh(np.asarray(devices), ("core",))
    n_outs = len(out_names)
    sharded = jax.jit(
        shard_map(
            _body,
            mesh=mesh,
            in_specs=(PartitionSpec("core"),) * (n_params + n_outs),
            out_specs=(PartitionSpec("core"),) * n_outs,
            check_rep=False,
        ),
        keep_unused=True,
    )
    concat_zeros = [
        np.zeros((NCORES * z.shape[0], *z.shape[1:]), z.dtype) for z in zero_outs
    ]

    def run(in_maps):
        concat_in = [
            np.concatenate([np.asarray(in_maps[c][nm]) for c in range(NCORES)], axis=0)
            for nm in in_names
        ]
        out_arrs = sharded(*concat_in, *concat_zeros)
        # per-core [512, H]; rows are 4 slots of 128 owner-remapped tokens
        return np.asarray(out_arrs[out_names.index("out")])

    _PROGRAM_CACHE["runner"] = run
    _PROGRAM_CACHE["runner_parts"] = (sharded, in_names, out_names, concat_zeros, mesh)
    return run


def _rope_tables():
    inv_freq = 1.0 / (ROPE_BASE ** (np.arange(0, ROTARY_DIM, 2, dtype=np.float64) / ROTARY_DIM))
    t = np.arange(S, dtype=np.float64)
    freqs = np.einsum("s,d->sd", t, inv_freq)          # [S, 16]
    emb = np.concatenate([freqs, freqs], axis=-1)       # [S, 32]
    cos = np.cos(emb).T.astype(np.float16)              # [32, S]
    sin = np.sin(emb).T.astype(np.float16)
    cosT = np.tile(cos, (1, B))                         # [32, T]  (batch-tiled)
    sinT = np.tile(sin, (1, B))
    return np.ascontiguousarray(cosT), np.ascontiguousarray(sinT)


def kernel(hidden_states, w_qkv, b_qkv, w_out, b_out):
    import ml_dtypes

    hidden_states = np.asarray(hidden_states, dtype=np.float32)
    w_qkv = np.asarray(w_qkv, dtype=np.float32)
    b_qkv = np.asarray(b_qkv, dtype=np.float32)
    w_out = np.asarray(w_out, dtype=np.float32)
    b_out = np.asarray(b_out, dtype=np.float32)

    xT = np.ascontiguousarray(hidden_states.reshape(T, HIDDEN).T.astype(np.float16))
    cosT, sinT = _rope_tables()
    # additive causal mask in [k, q] orientation: valid where q >= k
    r = np.arange(128)
    trim = np.where(r[None, :] >= r[:, None], 0.0, NEG_BIG).astype(np.float32)
    sgn_host = np.concatenate([-np.ones(16, np.float16), np.ones(16, np.float16)]).reshape(ROTARY_DIM, 1)
    wout_c = np.ascontiguousarray(w_out.astype(np.float16))

    in_maps = []
    for core in range(NCORES):
        hs = [HPC * core + j for j in range(HPC)]
        wq_i = np.concatenate([w_qkv[:, h * 384:h * 384 + 128] for h in hs], axis=1)
        wk_i = np.concatenate([w_qkv[:, h * 384 + 128:h * 384 + 256] for h in hs], axis=1)
        wv_i = np.concatenate([w_qkv[:, h * 384 + 256:h * 384 + 384] for h in hs], axis=1)
        in_maps.append({
            "xT": xT,
            "sgnd": sgn_host,
            "onekd": np.ones((128, 1), ml_dtypes.bfloat16),
            "wq": np.ascontiguousarray(wq_i.astype(np.float16)),
            "wk": np.ascontiguousarray(wk_i.astype(np.float16)),
            "wv": np.ascontiguousarray(wv_i.astype(np.float16)),
            "wout": wout_c,
            "cosd": cosT,
            "sind": sinT,
            "trid": trim,
        })

    out_cores = _runner()(in_maps)          # [8*512, H]

    # undo the owner remap: core c, slot g, row t holds global token
    # (g//2)*S + (g%2)*1024 + c*128 + t
    arr = out_cores.reshape(NCORES, NG, 128, HIDDEN)
    out_full = np.empty((T, HIDDEN), np.float32)
    for c in range(NCORES):
        for g in range(NG):
            s0 = (g // 2) * S + (g % 2) * 1024 + c * 128
            out_full[s0:s0 + 128] = arr[c, g]

    # exact host-side correction for the biases the device ignores:
    # v-bias contributes (softmax rows sum to 1): b_v @ w_out ; plus b_out.
    b_v = np.concatenate([b_qkv[h * 384 + 256:h * 384 + 384] for h in range(NUM_HEADS)])
    corr = b_v.astype(np.float64) @ w_out.astype(np.float64) + b_out.astype(np.float64)
    out_full = out_full + corr.astype(np.float32)[None, :]

    return out_full.reshape(B, S, HIDDEN)


# revision 47
# speedup vs baseline: 1.0213x; 1.0013x over previous
"""GPT-NeoX attention block on 8 Trainium2 NeuronCores (Bass/Tile).

Sharding: tensor-parallel over heads (16 heads -> 2 per core). Each core:
  - projects its 2 heads' q,k (feature-major) and v (token-major) from the
    full hidden states (fp16 matmuls: same 11-bit mantissa as tf32 but
    1 cycle/row at every free size, and half the DMA bytes),
  - applies partial RoPE (rotary_dim=32) to q,k per 512-token pair,
  - causal attention is INTERLEAVED into the projection loop: as soon as a
    512-token pair is RoPE'd, the four newly-enabled 128-token q-chunks run
    (scores k-major so softmax sums run on the PE via ones-matmuls; exp in
    bf16 for f32-range since scores are not max-subtracted). The PE therefore
    always has projection work to hide the score->exp->pv latency chain.
  - Output tokens are owner-remapped so AllToAll groups complete EVENLY
    through the run: group g = batch g//2, chunk-range (g%2)*8..(g%2)*8+7;
    dest core c takes the chunk with c2 % 8 == c of each group. All four
    collectives complete while projection work still runs; the host undoes
    the remap.
  - Each group's output projection (fp16, w_out fully resident) follows its
    collective at the end of the PE stream.
Host reassembles the scattered token slices and adds the bias correction.
"""
import sys

sys.path.insert(0, "/opt/trn_rl_repo")

import numpy as np

import concourse.bass as bass
import concourse.tile as tile
from concourse import bacc, mybir

# ---------------------------------------------------------------- constants
NUM_HEADS = 16
HIDDEN = 2048
HEAD_DIM = 128
ROTARY_DIM = 32
ROPE_BASE = 10000.0
B, S = 2, 2048
T = B * S                      # 4096 tokens
NCORES = 8
HPC = NUM_HEADS // NCORES      # 2 heads per core
W1 = 256                       # projection token-chunk width
NQ = S // 128                  # 16 q-chunks of 128 per batch
NG = 4                         # a2a groups: g = 2*b + c2//8
NEG_BIG = -30000.0             # causal-mask additive constant (exp -> 0)

f32 = mybir.dt.float32
f16 = mybir.dt.float16
bf16 = mybir.dt.bfloat16

_PROGRAM_CACHE = {}


def _build_program():
    """Build the SPMD Bass program (identical on all 8 cores)."""
    nc = bacc.Bacc(num_devices=NCORES, dynamic_dma_scratch_size=4096)

    xT = nc.dram_tensor("xT", [HIDDEN, T], f16, kind="ExternalInput")
    wq = nc.dram_tensor("wq", [HIDDEN, HPC * HEAD_DIM], f16, kind="ExternalInput")
    wk = nc.dram_tensor("wk", [HIDDEN, HPC * HEAD_DIM], f16, kind="ExternalInput")
    wv = nc.dram_tensor("wv", [HIDDEN, HPC * HEAD_DIM], f16, kind="ExternalInput")
    wout = nc.dram_tensor("wout", [HIDDEN, HIDDEN], f16, kind="ExternalInput")
    cosd = nc.dram_tensor("cosd", [ROTARY_DIM, T], f16, kind="ExternalInput")
    sind = nc.dram_tensor("sind", [ROTARY_DIM, T], f16, kind="ExternalInput")
    trid = nc.dram_tensor("trid", [128, 128], f32, kind="ExternalInput")
    sgnd = nc.dram_tensor("sgnd", [ROTARY_DIM, 1], f16, kind="ExternalInput")
    onekd = nc.dram_tensor("onekd", [128, 1], bf16, kind="ExternalInput")
    out = nc.dram_tensor("out", [T // NCORES, HIDDEN], f32, kind="ExternalOutput")

    KC = HIDDEN // 128          # 16 contraction chunks
    NCH = T // W1               # 16 projection token chunks
    shuffle_mask = [(i + 16) % 32 for i in range(32)]

    with tile.TileContext(nc) as tc:
        import contextlib

        with contextlib.ExitStack() as ctx:
            persist = ctx.enter_context(tc.tile_pool(name="persist", bufs=1))
            dram = ctx.enter_context(tc.tile_pool(name="dram", bufs=1, space="DRAM"))
            qkvpool = ctx.enter_context(tc.tile_pool(name="qkvpool", bufs=1))

            qT = qkvpool.tile([128, HPC, T], f16, name="qT", tag="qT")
            kT = qkvpool.tile([128, HPC, T], f16, name="kT", tag="kT")
            # token-major V: [tp, tt, c]; t = tt*128+tp, c = head*128+d
            vtm = qkvpool.tile([128, T // 128, HPC * HEAD_DIM], bf16, name="vtm", tag="vtm")
            tri = persist.tile([128, 128], f32, name="tri", tag="tri")
            sgn = persist.tile([32, 1], f16, name="sgn", tag="sgn")
            ones_k = persist.tile([128, 1], bf16, name="ones_k", tag="ones_k")

            # per-group AllToAll buffers: [dest, 2 heads' features, 128 tok]
            a2a_in = [
                dram.tile([NCORES, HPC * HEAD_DIM, 128], f16, name=f"a2a_in{g}", tag=f"a2a_in{g}")
                for g in range(NG)
            ]
            a2a_out = [
                dram.tile([NCORES, HPC * HEAD_DIM, 128], f16, name=f"a2a_out{g}", tag=f"a2a_out{g}")
                for g in range(NG)
            ]

            # w_out fully resident (fp16, 8MB) on the right SBUF edge
            woE = ctx.enter_context(tc.tile_pool(name="woE", bufs=1, side="right"))

            # -------------------------------- fused projection + attention
            with contextlib.ExitStack() as p1:
                wpool = p1.enter_context(tc.tile_pool(name="wpool", bufs=1))
                xpool = p1.enter_context(tc.tile_pool(name="xpool", bufs=3))
                rpool = p1.enter_context(tc.tile_pool(name="rpool", bufs=6))
                apool = p1.enter_context(tc.tile_pool(name="apool", bufs=8))
                ptpool = p1.enter_context(tc.tile_pool(name="ptpool", bufs=4))
                ps_p1 = p1.enter_context(tc.tile_pool(name="ps_p1", bufs=2, space="PSUM"))
                ps_s = p1.enter_context(tc.tile_pool(name="ps_s", bufs=4, space="PSUM"))
                ps_pv = p1.enter_context(tc.tile_pool(name="ps_pv", bufs=1, space="PSUM"))
                ps_l = p1.enter_context(tc.tile_pool(name="ps_l", bufs=1, space="PSUM"))

                wq_sb = wpool.tile([128, KC, HPC * HEAD_DIM], f16, name="wq_sb", tag="wq_sb")
                wk_sb = wpool.tile([128, KC, HPC * HEAD_DIM], f16, name="wk_sb", tag="wk_sb")
                wv_sb = wpool.tile([128, KC, HPC * HEAD_DIM], f16, name="wv_sb", tag="wv_sb")
                cos_sb = wpool.tile([ROTARY_DIM, T], f16, name="cos_sb", tag="cos_sb")
                sin_sb = wpool.tile([ROTARY_DIM, T], f16, name="sin_sb", tag="sin_sb")

                xT_r = xT[:].rearrange("(kc kp) t -> kp kc t", kp=128)

                # DMA issue order = model scheduling order: first q weights and
                # the first x chunk (gates the first matmul group), then the
                # remaining weights/tables. Later x chunks are prefetched one
                # iteration ahead inside the n loop; the w_out prefetch is
                # issued after the n loop.
                wq_r = wq[:].rearrange("(kc kp) c -> kp kc c", kp=128)
                xn0 = xpool.tile([128, KC, W1], f16, name="xn0", tag="xn")
                for gq in range(4):
                    kcs = slice(4 * gq, 4 * (gq + 1))
                    nc.sync.dma_start(out=wq_sb[:, kcs, :], in_=wq_r[:, kcs, :])
                    nc.gpsimd.dma_start(out=xn0[:, kcs, :], in_=xT_r[:, kcs, 0:W1])
                wk_r = wk[:].rearrange("(kc kp) c -> kp kc c", kp=128)
                wv_r = wv[:].rearrange("(kc kp) c -> kp kc c", kp=128)
                for gq in range(4):
                    kcs = slice(4 * gq, 4 * (gq + 1))
                    nc.sync.dma_start(out=wk_sb[:, kcs, :], in_=wk_r[:, kcs, :])
                    nc.sync.dma_start(out=wv_sb[:, kcs, :], in_=wv_r[:, kcs, :])
                xtiles = [xn0, xpool.tile([128, KC, W1], f16, name="xn1", tag="xn")]
                nc.sync.dma_start(out=xtiles[1][:], in_=xT_r[:, :, W1:2 * W1])
                nc.sync.dma_start(out=cos_sb[:], in_=cosd[:])
                nc.sync.dma_start(out=sin_sb[:], in_=sind[:])
                nc.sync.dma_start(out=tri[:], in_=trid[:])
                nc.sync.dma_start(out=sgn[:], in_=sgnd[:])
                nc.sync.dma_start(out=ones_k[:], in_=onekd[:])

                # ---- attention chunk machinery (software-pipelined) ----
                STAG = 3
                pending = []
                pair_state = {"n": 0, "ppv": None, "pl": None}

                def flush_one():
                    it = pending.pop(0)
                    ck = it["chunk"]
                    b, h, c2, nkb = ck["b"], ck["h"], ck["c2"], ck["c2"] + 1
                    hoff = ck["half"] * 128
                    for i, kb in enumerate(it["kbs"]):
                        nc.tensor.matmul(
                            ck["ppv"][:, hoff:hoff + 128],
                            vtm[:, b * NQ + kb, h * 128:(h + 1) * 128],
                            it["pt"][:, 128 * i:128 * (i + 1)],
                            start=(kb == 0), stop=(kb == nkb - 1),
                            skip_group_check=True,
                        )
                        nc.tensor.matmul(
                            ck["pl"][:, hoff:hoff + 128],
                            ones_k[:], it["pt"][:, 128 * i:128 * (i + 1)],
                            start=(kb == 0), stop=(kb == nkb - 1),
                            skip_group_check=True,
                        )
                    if it["last"]:
                        tag = f"{b}{h}{c2}"
                        l_sb = apool.tile([1, 128], f32, name=f"l{tag}", tag="l_sb")
                        nc.vector.tensor_copy(out=l_sb[:], in_=ck["pl"][:, hoff:hoff + 128])
                        lbc = apool.tile([128, 128], f32, name=f"lbc{tag}", tag="lbc")
                        nc.gpsimd.partition_broadcast(lbc[:], l_sb[:])
                        recip = apool.tile([128, 128], f32, name=f"rc{tag}", tag="recip")
                        nc.vector.reciprocal(out=recip[:], in_=lbc[:])
                        attn_sb = apool.tile([128, 128], f16, name=f"at{tag}", tag="attn_sb")
                        nc.vector.tensor_mul(attn_sb[:], ck["ppv"][:, hoff:hoff + 128], recip[:])
                        g = 2 * b + c2 // 8
                        nc.sync.dma_start(
                            out=a2a_in[g][c2 % 8, h * 128:(h + 1) * 128, :],
                            in_=attn_sb[:],
                        )

                def emit_chunk(b, h, c2):
                    nkb = c2 + 1
                    qcol = slice(b * S + c2 * 128, b * S + (c2 + 1) * 128)
                    if pair_state["n"] % 4 == 0:
                        pair_state["ppv"] = ps_pv.tile([128, 512], f32, name=f"ppv{b}{h}{c2}", tag="ppv")
                        pair_state["pl"] = ps_l.tile([1, 512], f32, name=f"pl{b}{h}{c2}", tag="pl")
                    chunk = {
                        "b": b, "h": h, "c2": c2,
                        "half": pair_state["n"] % 4,
                        "ppv": pair_state["ppv"],
                        "pl": pair_state["pl"],
                    }
                    pair_state["n"] += 1
                    kb0 = 0
                    while kb0 < nkb:
                        nt = min(4, nkb - kb0)
                        while len(pending) >= STAG:
                            flush_one()
                        ps = ps_s.tile([128, nt * 128], f32, name=f"ps{b}{h}{c2}{kb0}", tag="ps")
                        pt = ptpool.tile([128, nt * 128], bf16, name=f"pt{b}{h}{c2}{kb0}", tag="pt")
                        for i in range(nt):
                            kb = kb0 + i
                            kcol = slice(b * S + kb * 128, b * S + (kb + 1) * 128)
                            nc.tensor.matmul(
                                ps[:, 128 * i:128 * (i + 1)],
                                kT[:, h, kcol], qT[:, h, qcol],
                                start=True, stop=True,
                            )
                            if kb == c2:
                                nc.vector.tensor_add(
                                    ps[:, 128 * i:128 * (i + 1)],
                                    ps[:, 128 * i:128 * (i + 1)],
                                    tri[:],
                                )
                        nc.scalar.activation(
                            out=pt[:], in_=ps[:],
                            func=mybir.ActivationFunctionType.Exp,
                        )
                        pending.append({
                            "chunk": chunk, "pt": pt,
                            "kbs": list(range(kb0, kb0 + nt)),
                            "last": kb0 + nt == nkb,
                        })
                        kb0 += nt

                # ---- fused projection/attention loop ----
                for n in range(NCH):
                    tcol = slice(n * W1, (n + 1) * W1)
                    xn = xtiles[n]
                    if 1 <= n < NCH - 1:
                        xt = xpool.tile([128, KC, W1], f16, name=f"xn{n+1}", tag="xn")
                        nc.sync.dma_start(out=xt[:], in_=xT_r[:, :, slice((n + 1) * W1, (n + 2) * W1)])
                        xtiles.append(xt)

                    # q/k feature-major: psum[c, t] += w[k, c].T @ x[k, t]
                    for ct in range(4):
                        w_sb = wq_sb if ct < 2 else wk_sb
                        h = ct % 2
                        tgt = qT if ct < 2 else kT
                        pqk = ps_p1.tile([128, W1], f32, name=f"pqk{n}_{ct}", tag="p1")
                        for kc in range(KC):
                            nc.tensor.matmul(
                                pqk[:],
                                w_sb[:, kc, h * 128:(h + 1) * 128],
                                xn[:, kc, :],
                                start=(kc == 0),
                                stop=(kc == KC - 1),
                            )
                        nc.scalar.copy(out=tgt[:, h, tcol], in_=pqk[:])

                    # v token-major: psum[t, c] += x[k, t].T @ wv[k, c]
                    for t2 in range(W1 // 128):
                        pv = ps_p1.tile([128, HPC * HEAD_DIM], f32, name=f"pv{n}_{t2}", tag="p1")
                        for kc in range(KC):
                            nc.tensor.matmul(
                                pv[:],
                                xn[:, kc, t2 * 128:(t2 + 1) * 128],
                                wv_sb[:, kc, :],
                                start=(kc == 0),
                                stop=(kc == KC - 1),
                            )
                        nc.scalar.copy(out=vtm[:, n * (W1 // 128) + t2, :], in_=pv[:])

                    if n % 2 == 1:
                        # RoPE on the rotary rows of this 512-token pair
                        seg = slice((n - 1) * W1, (n + 1) * W1)
                        for tgt in (qT, kT):
                            for h in range(HPC):
                                shuf = rpool.tile([32, 2 * W1], f16, name=f"shuf{n}_{h}", tag="shuf")
                                nc.vector.stream_shuffle(shuf[:], tgt[0:32, h, seg], shuffle_mask)
                                nc.vector.scalar_tensor_tensor(
                                    out=shuf[:],
                                    in0=shuf[:],
                                    scalar=sgn[:, 0:1],
                                    in1=sin_sb[:, seg],
                                    op0=mybir.AluOpType.mult,
                                    op1=mybir.AluOpType.mult,
                                )
                                nc.vector.tensor_mul(tgt[0:32, h, seg], tgt[0:32, h, seg], cos_sb[:, seg])
                                nc.vector.tensor_add(tgt[0:32, h, seg], tgt[0:32, h, seg], shuf[:])

                        # four q-chunks newly enabled by this pair's RoPE
                        b = n // 8
                        lo = 2 * ((n % 8) - 1)
                        for c2 in range(lo, lo + 4):
                            for h in range(HPC):
                                emit_chunk(b, h, c2)
                        if n % 8 in (3, 7):
                            # a2a group complete: g = 2*b + (n%8)//4
                            g = 2 * b + (n % 8) // 4
                            while pending:
                                flush_one()
                            nc.gpsimd.collective_compute(
                                "AllToAll",
                                mybir.AluOpType.bypass,
                                replica_groups=[list(range(NCORES))],
                                ins=[a2a_in[g].opt()],
                                outs=[a2a_out[g].opt()],
                            )

                wo_sb = []
                for dc in range(KC):
                    wt = woE.tile([128, HIDDEN], f16, name=f"wo{dc}", tag=f"wo{dc}")
                    nc.sync.dma_start(out=wt[:], in_=wout[dc * 128:(dc + 1) * 128, :])
                    wo_sb.append(wt)

            # ---------------------------------------------- output projection
            # group g supplies this core's out rows [g*128, (g+1)*128)
            with contextlib.ExitStack() as p3:
                atpool = p3.enter_context(tc.tile_pool(name="atpool", bufs=2, side="right"))
                opool = p3.enter_context(tc.tile_pool(name="opool", bufs=2, side="right"))
                ps_o = p3.enter_context(tc.tile_pool(name="ps_o", bufs=2, space="PSUM"))

                for g in range(NG):
                    # attnT loads slot into the SP queue well after their
                    # collective completes (pseudo-timestamps steer only the
                    # Tile scheduler's placement; the cost model ignores them)
                    attnT = atpool.tile([128, KC, 128], f16, name=f"attnT{g}", tag="attnT")
                    with tc.tile_wait_until([0.17, 0.23, 0.5, 0.52][g]):
                        nc.sync.dma_start(
                            out=attnT[:],
                            in_=a2a_out[g][:]
                            .rearrange("s q t -> (s q) t")
                            .rearrange("(dc dp) t -> dp dc t", dp=128),
                        )
                    with tc.tile_wait_until(0.6 + 0.02 * g):
                        osb = opool.tile([128, HIDDEN], f32, name=f"osb{g}", tag="osb")
                        for oc in range(4):
                            po = ps_o.tile([128, 512], f32, name=f"po{g}{oc}", tag="po")
                            for dc in range(KC):
                                nc.tensor.matmul(
                                    po[:],
                                    attnT[:, dc, :],
                                    wo_sb[dc][:, oc * 512:(oc + 1) * 512],
                                    start=(dc == 0),
                                    stop=(dc == KC - 1),
                                )
                            nc.scalar.copy(out=osb[:, oc * 512:(oc + 1) * 512], in_=po[:])
                            nc.sync.dma_start(
                                out=out[g * 128:(g + 1) * 128, oc * 512:(oc + 1) * 512],
                                in_=osb[:, oc * 512:(oc + 1) * 512],
                            )

    nc.finalize()
    return nc


def _runner():
    """Build (once) a reusable jitted SPMD executor over the 8 cores.

    Returns a callable: in_maps (list of per-core dicts) -> per-core outputs.
    """
    if "runner" in _PROGRAM_CACHE:
        return _PROGRAM_CACHE["runner"]

    import jax
    from jax.sharding import Mesh, PartitionSpec
    try:
        from jax.experimental.shard_map import shard_map
    except Exception:
        from jax.shard_map import shard_map  # newer jax
    from concourse import bass2jax
    from concourse.bass2jax import _bass_exec_p, partition_id_tensor, install_neuronx_cc_hook

    install_neuronx_cc_hook()
    nc = _build_program()
    _PROGRAM_CACHE["nc"] = nc

    partition_name = nc.partition_id_tensor.name if nc.partition_id_tensor else None
    in_names, out_names, out_avals, zero_outs = [], [], [], []
    for alloc in nc.m.functions[0].allocations:
        if not isinstance(alloc, mybir.MemoryLocationSet):
            continue
        name = alloc.memorylocations[0].name
        if alloc.kind == "ExternalInput":
            if name != partition_name:
                in_names.append(name)
        elif alloc.kind == "ExternalOutput":
            out_names.append(name)
            shape = tuple(alloc.tensor_shape)
            dtype = mybir.dt.np(alloc.dtype)
            out_avals.append(jax.core.ShapedArray(shape, dtype))
            zero_outs.append(np.zeros(shape, dtype))
    n_params = len(in_names)
    all_in_names = list(in_names) + list(out_names)
    if partition_name is not None:
        all_in_names.append(partition_name)

    def _body(*args):
        operands = list(args)
        if partition_name is not None:
            operands.append(partition_id_tensor())
        outs = _bass_exec_p.bind(
            *operands,
            out_avals=tuple(out_avals),
            in_names=tuple(all_in_names),
            out_names=tuple(out_names),
            lowering_input_output_aliases=(),
            sim_require_finite=True,
            sim_require_nnan=True,
            nc=nc,
        )
        return tuple(outs)

    devices = jax.devices()[:NCORES]
    mesh = Mesh(np.asarray(devices), ("core",))
    n_outs = len(out_names)
    sharded = jax.jit(
        shard_map(
            _body,
            mesh=mesh,
            in_specs=(PartitionSpec("core"),) * (n_params + n_outs),
            out_specs=(PartitionSpec("core"),) * n_outs,
            check_rep=False,
        ),
        keep_unused=True,
    )
    concat_zeros = [
        np.zeros((NCORES * z.shape[0], *z.shape[1:]), z.dtype) for z in zero_outs
    ]

    def run(in_maps):
        concat_in = [
            np.concatenate([np.asarray(in_maps[c][nm]) for c in range(NCORES)], axis=0)
            for nm in in_names
        ]
        out_arrs = sharded(*concat_in, *concat_zeros)
        # per-core [512, H]; rows are 4 slots of 128 owner-remapped tokens
        return np.asarray(out_arrs[out_names.index("out")])

    _PROGRAM_CACHE["runner"] = run
    _PROGRAM_CACHE["runner_parts"] = (sharded, in_names, out_names, concat_zeros, mesh)
    return run


def _rope_tables():
    inv_freq = 1.0 / (ROPE_BASE ** (np.arange(0, ROTARY_DIM, 2, dtype=np.float64) / ROTARY_DIM))
    t = np.arange(S, dtype=np.float64)
    freqs = np.einsum("s,d->sd", t, inv_freq)          # [S, 16]
    emb = np.concatenate([freqs, freqs], axis=-1)       # [S, 32]
    cos = np.cos(emb).T.astype(np.float16)              # [32, S]
    sin = np.sin(emb).T.astype(np.float16)
    cosT = np.tile(cos, (1, B))                         # [32, T]  (batch-tiled)
    sinT = np.tile(sin, (1, B))
    return np.ascontiguousarray(cosT), np.ascontiguousarray(sinT)


def kernel(hidden_states, w_qkv, b_qkv, w_out, b_out):
    import ml_dtypes

    hidden_states = np.asarray(hidden_states, dtype=np.float32)
    w_qkv = np.asarray(w_qkv, dtype=np.float32)
    b_qkv = np.asarray(b_qkv, dtype=np.float32)
    w_out = np.asarray(w_out, dtype=np.float32)
    b_out = np.asarray(b_out, dtype=np.float32)

    xT = np.ascontiguousarray(hidden_states.reshape(T, HIDDEN).T.astype(np.float16))
    cosT, sinT = _rope_tables()
    # additive causal mask in [k, q] orientation: valid where q >= k
    r = np.arange(128)
    trim = np.where(r[None, :] >= r[:, None], 0.0, NEG_BIG).astype(np.float32)
    sgn_host = np.concatenate([-np.ones(16, np.float16), np.ones(16, np.float16)]).reshape(ROTARY_DIM, 1)
    wout_c = np.ascontiguousarray(w_out.astype(np.float16))

    in_maps = []
    for core in range(NCORES):
        hs = [HPC * core + j for j in range(HPC)]
        wq_i = np.concatenate([w_qkv[:, h * 384:h * 384 + 128] for h in hs], axis=1)
        wk_i = np.concatenate([w_qkv[:, h * 384 + 128:h * 384 + 256] for h in hs], axis=1)
        wv_i = np.concatenate([w_qkv[:, h * 384 + 256:h * 384 + 384] for h in hs], axis=1)
        in_maps.append({
            "xT": xT,
            "sgnd": sgn_host,
            "onekd": np.ones((128, 1), ml_dtypes.bfloat16),
            "wq": np.ascontiguousarray(wq_i.astype(np.float16)),
            "wk": np.ascontiguousarray(wk_i.astype(np.float16)),
            "wv": np.ascontiguousarray(wv_i.astype(np.float16)),
            "wout": wout_c,
            "cosd": cosT,
            "sind": sinT,
            "trid": trim,
        })

    out_cores = _runner()(in_maps)          # [8*512, H]

    # undo the owner remap: core c, slot g, row t holds global token
    # (g//2)*S + (g%2)*1024 + c*128 + t
    arr = out_cores.reshape(NCORES, NG, 128, HIDDEN)
    out_full = np.empty((T, HIDDEN), np.float32)
    for c in range(NCORES):
        for g in range(NG):
            s0 = (g // 2) * S + (g % 2) * 1024 + c * 128
            out_full[s0:s0 + 128] = arr[c, g]

    # exact host-side correction for the biases the device ignores:
    # v-bias contributes (softmax rows sum to 1): b_v @ w_out ; plus b_out.
    b_v = np.concatenate([b_qkv[h * 384 + 256:h * 384 + 384] for h in range(NUM_HEADS)])
    corr = b_v.astype(np.float64) @ w_out.astype(np.float64) + b_out.astype(np.float64)
    out_full = out_full + corr.astype(np.float32)[None, :]

    return out_full.reshape(B, S, HIDDEN)
